# revision 1
# baseline (speedup 1.0000x reference)
"""Trainium2 Bass kernel for a 2-layer GraphNetwork (gnn_message_passing).

Strategy:
  - 16 graphs are partitioned across 8 cores (2 graphs per core). Every
    edge's receiver node lives on the edge's own core, so ALL segment
    reductions (per-node edge means, per-graph pooling) are core-local.
    No collectives are needed; the [16,128] output rows are gathered on
    the host.
  - Per core, nodes are bin-packed into NT tiles of 128 "slots"; each
    tile's incoming edges are padded to K0 chunks of 128. Segment-sums
    are computed on the tensor engine as one-hot selector matmuls
    (the one-hot [128e x 128n] block is built on-device from a column
    index via DVE is_equal against an iota tile).
  - Weights are replicated; biases are folded into matmuls via ones-rows.
  - bf16 inputs/intermediates, fp32 PSUM accumulation, fp32 final stage.
"""

import numpy as np
import ml_dtypes

import concourse.bass as bass
import concourse.tile as tile_mod
from concourse import tile
from concourse.bass_utils import run_bass_kernel_spmd
from concourse.vector_clock import ScopedClock

mybir = bass.mybir

N_NODES, N_EDGES, N_GRAPHS = 20000, 320000, 16
F_NODE, F_EDGE, F_GLOB = 64, 32, 16
N_CORES = 8
GPC = N_GRAPHS // N_CORES  # graphs per core = 2

BF16 = mybir.dt.bfloat16
F32 = mybir.dt.float32
npbf16 = ml_dtypes.bfloat16

# ---------------------------------------------------------------------------
# Workaround: CoreV3 codegen rejects the TileContext final drain when it
# carries more than one semaphore wait. Split the waits across extra no-ops.
_MAX_WAITS = 1


_ENGINE_WAIT_LIMIT = 1
_SPLIT_ENGINES = None  # set lazily


def _split_excess_waits(nc):
    """CoreV3 codegen caps per-instruction sem waits. Move excess waits
    onto same-engine no-ops inserted immediately before the offender."""
    global _SPLIT_ENGINES
    if _SPLIT_ENGINES is None:
        ET = mybir.EngineType
        _SPLIT_ENGINES = {ET.PE, ET.Activation, ET.DVE, ET.SP, ET.Pool}
    ctr = [0]
    for bass_bb in nc.bb_map.values():
        bb = bass_bb.bb
        il = bb.instructions
        out = []
        changed = False
        for inst in il:
            si = inst.sync_info
            waits = list(si.on_wait) if (si and si.on_wait) else []
            if len(waits) > _ENGINE_WAIT_LIMIT and inst.engine in _SPLIT_ENGINES:
                head, keep = waits[:-_ENGINE_WAIT_LIMIT], waits[-_ENGINE_WAIT_LIMIT:]
                for i in range(0, len(head), _ENGINE_WAIT_LIMIT):
                    nop = mybir.InstNoOp(name=f"waitsplit-{ctr[0]}", ins=[], outs=[])
                    ctr[0] += 1
                    nop.engine = inst.engine
                    nop.sync_info = mybir.SyncInfo(
                        on_wait=head[i : i + _ENGINE_WAIT_LIMIT], on_update=[]
                    )
                    nc.register_instruction(nop, overwrite=True)
                    out.append(nop)
                inst.sync_info = mybir.SyncInfo(
                    on_wait=keep, on_update=list(si.on_update or [])
                )
                changed = True
            out.append(inst)
        if changed:
            bb.instructions = out


def _split_drain_and_barrier(self, tick_clock, wait_clock):
    nc = self.nc
    _split_excess_waits(nc)
    drain_inst = nc.sync.drain()
    wait_clock.add_sem_waits(
        drain_inst.ins, ScopedClock({None: tick_clock.global_clock})
    )
    mi = drain_inst.ins
    waits = list(mi.sync_info.on_wait) if (mi.sync_info and mi.sync_info.on_wait) else []
    if len(waits) > _MAX_WAITS:
        upd = list(mi.sync_info.on_update) if mi.sync_info.on_update else []
        mi.sync_info = mybir.SyncInfo(on_wait=waits[:_MAX_WAITS], on_update=upd)
        for i in range(_MAX_WAITS, len(waits), _MAX_WAITS):
            nop = nc.sync.nop(nofuse=True)
            nop.ins.sync_info = mybir.SyncInfo(
                on_wait=waits[i : i + _MAX_WAITS], on_update=[]
            )
    nc.all_engine_barrier()
    assert self.sems is not None
    popped = nc._tile_sem_poison_stack.pop()
    assert popped is self._sem_poison
    nc.clear_and_free_semaphores(list(self.sems.allocated().values()))
    nc.all_engine_barrier()


tile_mod.TileContext._drain_and_barrier = _split_drain_and_barrier


# ---------------------------------------------------------------------------
# Host-side graph partitioning / layout


def _pack_core(node_ids, degs, nt, cap_e):
    """LPT: place nodes (descending degree) onto the least-edge-loaded tile
    that still has node capacity. Returns per-tile node-id arrays, or None
    if some tile exceeds cap_e edges."""
    order = np.argsort(-degs, kind="stable")
    tiles_n = [[] for _ in range(nt)]
    tile_ncnt = np.zeros(nt, np.int64)
    tile_ecnt = np.zeros(nt, np.int64)
    for j in order:
        cand = np.where(tile_ncnt < 128)[0]
        if len(cand) == 0:
            return None
        t = cand[np.argmin(tile_ecnt[cand])]
        tiles_n[t].append(node_ids[j])
        tile_ncnt[t] += 1
        tile_ecnt[t] += degs[j]
    if (tile_ecnt > cap_e).any():
        return None
    return [np.array(t, dtype=np.int64) for t in tiles_n]


def _prepare(inputs):
    nf = np.asarray(inputs["node_feats"], np.float32)
    ef = np.asarray(inputs["edge_feats"], np.float32)
    glob = np.asarray(inputs["globals_"], np.float32)
    recv = np.asarray(inputs["receivers"]).astype(np.int64)
    ngraph = np.asarray(inputs["node_graph"]).astype(np.int64)

    cnt = np.bincount(recv, minlength=N_NODES).astype(np.int64)
    egraph = ngraph[recv]
    ncnt_g = np.bincount(ngraph, minlength=N_GRAPHS)
    ecnt_g = np.bincount(egraph, minlength=N_GRAPHS)

    node_core = ngraph // GPC
    edge_core = egraph // GPC

    core_nodes = [np.where(node_core == c)[0] for c in range(N_CORES)]
    NT = int(max((len(cn) + 127) // 128 for cn in core_nodes))

    packs = None
    K0 = max(1, int(max(np.bincount(edge_core, minlength=N_CORES)) + NT * 128 - 1)
             // (NT * 128))
    for k0 in range(K0, K0 + 12):
        trial = []
        ok = True
        for c in range(N_CORES):
            p = _pack_core(core_nodes[c], cnt[core_nodes[c]], NT, k0 * 128)
            if p is None:
                ok = False
                break
            trial.append(p)
        if ok:
            packs, K0 = trial, k0
            break
    assert packs is not None, "bin packing failed"

    NPAD = NT * 128
    EPAD = NT * K0 * 128

    # slot assignment per core
    w_np = {}
    slot_of_node = np.full(N_NODES, -1, np.int64)
    tile_of_node = np.full(N_NODES, -1, np.int64)
    in_maps = []
    for c in range(N_CORES):
        for t in range(NT):
            ids = packs[c][t]
            slot_of_node[ids] = t * 128 + np.arange(len(ids))
            tile_of_node[ids] = t

        # ---- edges
        eidx = np.where(edge_core == c)[0]
        et = tile_of_node[recv[eidx]]
        order = np.argsort(et, kind="stable")
        eidx = eidx[order]
        et = et[order]
        counts = np.bincount(et, minlength=NT)
        starts = np.concatenate([[0], np.cumsum(counts)[:-1]])
        off_in = np.arange(len(eidx)) - np.repeat(starts, counts)
        dst = et * (K0 * 128) + off_in
        assert (counts <= K0 * 128).all()

        eftT = np.zeros((33, EPAD), np.float32)
        eftT[:32, dst] = ef[eidx].T
        eftT[32, dst] = 1.0

        eg_loc = egraph[eidx] - c * GPC
        ghot = np.zeros((3, EPAD), np.float32)
        ghot[0, dst] = (eg_loc == 0)
        ghot[1, dst] = (eg_loc == 1)
        ghot[2, dst] = 1.0

        selidx = np.full(EPAD, -1.0, np.float32)
        selidx[dst] = (slot_of_node[recv[eidx]] % 128).astype(np.float32)
        # [NT, 128, K0] : chunk k, lane i  <- position (t*K0 + k)*128 + i
        sel3 = selidx.reshape(NT, K0, 128).transpose(0, 2, 1).copy()

        # ---- nodes
        slot_node = np.full(NPAD, -1, np.int64)
        for t in range(NT):
            ids = packs[c][t]
            slot_node[t * 128 : t * 128 + len(ids)] = ids
        valid = slot_node >= 0
        sn = np.where(valid, slot_node, 0)

        nftT = np.zeros((65, NPAD), np.float32)
        nftT[:64, valid] = nf[sn[valid]].T
        nftT[64, valid] = 1.0

        ng_loc = ngraph[sn] - c * GPC
        nhot = np.zeros((3, NPAD), np.float32)
        nhot[0] = valid * (ng_loc == 0)
        nhot[1] = valid * (ng_loc == 1)
        nhot[2] = valid * 1.0

        invc = np.zeros((NPAD, 1), np.float32)
        invc[valid, 0] = 1.0 / np.maximum(cnt[sn[valid]], 1)

        poolw = np.zeros((NPAD, 4), np.float32)
        for g in range(GPC):
            gid = c * GPC + g
            m = valid & (ng_loc == g)
            poolw[m, g] = 1.0 / max(ncnt_g[gid], 1)
            poolw[m, 2 + g] = cnt[sn[m]] / max(ecnt_g[gid], 1)

        globT = glob[c * GPC : (c + 1) * GPC].T.copy()  # [16, 2]

        in_maps.append(
            {
                "eft": eftT.astype(npbf16),
                "ghot": ghot.astype(npbf16),
                "selidx": sel3,
                "nft": nftT.astype(npbf16),
                "nhot": nhot.astype(npbf16),
                "invc": invc,
                "poolw": poolw.astype(npbf16),
                "globT": globT,
            }
        )

    # ---- replicated weights
    def bf(x):
        return np.ascontiguousarray(x).astype(npbf16)

    We1T = np.zeros((33, 256), np.float32)
    We1T[:32] = np.asarray(inputs["We1"], np.float32).T
    We1T[32] = np.asarray(inputs["be1"], np.float32)
    w_np["We1T"] = bf(We1T)

    We2 = np.asarray(inputs["We2"], np.float32)  # [128, 256]
    We2T = We2.T  # [256, 128]
    w_np["We2T"] = bf(np.concatenate([We2T[:128], We2T[128:]], axis=1))  # [128, 256]

    Wn1T = np.zeros((65, 256), np.float32)
    Wn1T[:64] = np.asarray(inputs["Wn1"], np.float32).T
    Wn1T[64] = np.asarray(inputs["bn1"], np.float32)
    w_np["Wn1T"] = bf(Wn1T)

    Win1T = np.asarray(inputs["Win1"], np.float32).T  # [256 fi, 256 fo]
    w_np["Win1T"] = bf(
        np.concatenate(
            [Win1T[:128, :128], Win1T[:128, 128:], Win1T[128:, :128], Win1T[128:, 128:]],
            axis=1,
        )
    )  # [128, 512] : cols b*256 + s*128

    Wn2T = np.asarray(inputs["Wn2"], np.float32).T  # [256, 128]
    w_np["Wn2T"] = bf(np.concatenate([Wn2T[:128], Wn2T[128:]], axis=1))  # [128, 256]
    w_np["Win2T"] = bf(np.asarray(inputs["Win2"], np.float32).T)  # [128, 128]

    w_np["Wg2T"] = np.asarray(inputs["Wg2"], np.float32).T.copy()  # [16, 128] f32
    w_np["Wng2T"] = np.asarray(inputs["Wng2"], np.float32).T.copy()
    w_np["be2r"] = bf(np.asarray(inputs["be2"], np.float32)[None, :])
    w_np["bn2r"] = bf(np.asarray(inputs["bn2"], np.float32)[None, :])

    w_np["WgnT"] = np.asarray(inputs["Wgn"], np.float32).T.copy()  # [128,128] f32
    w_np["WgeT"] = np.asarray(inputs["Wge"], np.float32).T.copy()
    w_np["WggT"] = np.asarray(inputs["Wgg"], np.float32).T.copy()  # [16, 128]
    w_np["bgr"] = np.asarray(inputs["bg"], np.float32)[None, :].copy()
    w_np["ones2"] = np.ones((1, 2), np.float32)
    w_np["iota"] = np.broadcast_to(
        np.arange(128, dtype=np.float32), (128, 128)
    ).copy()
    w_np["ident"] = np.eye(128, dtype=npbf16)
    w_np["ident2"] = np.eye(2, dtype=np.float32)

    for m in in_maps:
        m.update(w_np)
    return in_maps, NT, K0


# ---------------------------------------------------------------------------
# Device program (identical on all cores)


def _build(NT, K0):
    nc = bass.Bass()
    NPAD = NT * 128
    EPAD = NT * K0 * 128
    CW = K0 * 128  # edge columns per node-tile

    d_eft = nc.dram_tensor("eft", [33, EPAD], BF16, kind="ExternalInput")
    d_ghot = nc.dram_tensor("ghot", [3, EPAD], BF16, kind="ExternalInput")
    d_sel = nc.dram_tensor("selidx", [NT, 128, K0], F32, kind="ExternalInput")
    d_nft = nc.dram_tensor("nft", [65, NPAD], BF16, kind="ExternalInput")
    d_nhot = nc.dram_tensor("nhot", [3, NPAD], BF16, kind="ExternalInput")
    d_invc = nc.dram_tensor("invc", [NPAD, 1], F32, kind="ExternalInput")
    d_poolw = nc.dram_tensor("poolw", [NPAD, 4], BF16, kind="ExternalInput")
    d_globT = nc.dram_tensor("globT", [16, 2], F32, kind="ExternalInput")

    d_We1T = nc.dram_tensor("We1T", [33, 256], BF16, kind="ExternalInput")
    d_We2T = nc.dram_tensor("We2T", [128, 256], BF16, kind="ExternalInput")
    d_Wn1T = nc.dram_tensor("Wn1T", [65, 256], BF16, kind="ExternalInput")
    d_Win1T = nc.dram_tensor("Win1T", [128, 512], BF16, kind="ExternalInput")
    d_Wn2T = nc.dram_tensor("Wn2T", [128, 256], BF16, kind="ExternalInput")
    d_Win2T = nc.dram_tensor("Win2T", [128, 128], BF16, kind="ExternalInput")
    d_Wg2T = nc.dram_tensor("Wg2T", [16, 128], F32, kind="ExternalInput")
    d_Wng2T = nc.dram_tensor("Wng2T", [16, 128], F32, kind="ExternalInput")
    d_be2r = nc.dram_tensor("be2r", [1, 128], BF16, kind="ExternalInput")
    d_bn2r = nc.dram_tensor("bn2r", [1, 128], BF16, kind="ExternalInput")
    d_WgnT = nc.dram_tensor("WgnT", [128, 128], F32, kind="ExternalInput")
    d_WgeT = nc.dram_tensor("WgeT", [128, 128], F32, kind="ExternalInput")
    d_WggT = nc.dram_tensor("WggT", [16, 128], F32, kind="ExternalInput")
    d_bgr = nc.dram_tensor("bgr", [1, 128], F32, kind="ExternalInput")
    d_ones2 = nc.dram_tensor("ones2", [1, 2], F32, kind="ExternalInput")
    d_iota = nc.dram_tensor("iota", [128, 128], F32, kind="ExternalInput")
    d_ident = nc.dram_tensor("ident", [128, 128], BF16, kind="ExternalInput")
    d_ident2 = nc.dram_tensor("ident2", [2, 2], F32, kind="ExternalInput")

    d_out = nc.dram_tensor("out", [128, 2], F32, kind="ExternalOutput")

    Relu = mybir.ActivationFunctionType.Relu
    Copy = mybir.ActivationFunctionType.Copy

    with tile.TileContext(nc) as tc:
        with tc.tile_pool(name="wp", bufs=1) as wp:
            def wtile(dram, shape, dt):
                t = wp.tile(shape, dt, tag=dram.name)
                nc.sync.dma_start(t[:], dram[:])
                return t

            We1T = wtile(d_We1T, [33, 256], BF16)
            We2T = wtile(d_We2T, [128, 256], BF16)
            Wn1T = wtile(d_Wn1T, [65, 256], BF16)
            Win1T = wtile(d_Win1T, [128, 512], BF16)
            Wn2T = wtile(d_Wn2T, [128, 256], BF16)
            Win2T = wtile(d_Win2T, [128, 128], BF16)
            Wg2T = wtile(d_Wg2T, [16, 128], F32)
            Wng2T = wtile(d_Wng2T, [16, 128], F32)
            WgnT = wtile(d_WgnT, [128, 128], F32)
            WgeT = wtile(d_WgeT, [128, 128], F32)
            WggT = wtile(d_WggT, [16, 128], F32)
            bgr = wtile(d_bgr, [1, 128], F32)
            ones2 = wtile(d_ones2, [1, 2], F32)
            iota = wtile(d_iota, [128, 128], F32)
            ident = wtile(d_ident, [128, 128], BF16)
            ident2 = wtile(d_ident2, [2, 2], F32)
            globT = wtile(d_globT, [16, 2], F32)

            aggall = wp.tile([128, 384 * NT], BF16, tag="aggall")
            g2aug = wp.tile([3, 128], BF16, tag="g2aug")
            gnaug = wp.tile([3, 128], BF16, tag="gnaug")

            # --- per-core global projections gb = globals @ Wg2.T etc.
            with tc.tile_pool(name="psg", bufs=1, space=bass.MemorySpace.PSUM) as psg:
                pg = psg.tile([2, 256], F32, tag="pg")
                nc.tensor.matmul(pg[:, 0:128], globT[:], Wg2T[:], start=True, stop=True)
                nc.tensor.matmul(pg[:, 128:256], globT[:], Wng2T[:], start=True, stop=True)
                nc.scalar.activation(g2aug[0:2, :], pg[:, 0:128], Copy)
                nc.scalar.activation(gnaug[0:2, :], pg[:, 128:256], Copy)
                nc.sync.dma_start(g2aug[2:3, :], d_be2r[:])
                nc.sync.dma_start(gnaug[2:3, :], d_bn2r[:])

            # ----------------- edge phase -----------------
            with tc.tile_pool(name="ep", bufs=3) as ep, \
                 tc.tile_pool(name="esb", bufs=6) as esb, \
                 tc.tile_pool(name="psA", bufs=2, space=bass.MemorySpace.PSUM) as psA, \
                 tc.tile_pool(name="psB", bufs=2, space=bass.MemorySpace.PSUM) as psB, \
                 tc.tile_pool(name="psC", bufs=2, space=bass.MemorySpace.PSUM) as psC, \
                 tc.tile_pool(name="psAgg", bufs=2, space=bass.MemorySpace.PSUM) as psAgg:
                for t in range(NT):
                    eftt = ep.tile([33, CW], BF16, tag="eftt")
                    nc.sync.dma_start(eftt[:], d_eft[:, t * CW : (t + 1) * CW])
                    ght = ep.tile([3, CW], BF16, tag="ght")
                    nc.sync.dma_start(ght[:], d_ghot[:, t * CW : (t + 1) * CW])
                    sidx = ep.tile([128, K0], F32, tag="sidx")
                    nc.sync.dma_start(sidx[:], d_sel[t])
                    invc_t = ep.tile([128, 1], F32, tag="invc")
                    nc.sync.dma_start(invc_t[:], d_invc[t * 128 : (t + 1) * 128, :])

                    # one-hot selector columns for the whole tile, up front
                    ohall = esb.tile([128, CW], BF16, tag="ohall", bufs=2)
                    for k in range(K0):
                        nc.vector.tensor_scalar(
                            ohall[:, k * 128 : (k + 1) * 128], iota[:],
                            sidx[:, k : k + 1], None,
                            op0=mybir.AluOpType.is_equal,
                        )

                    pagg = psAgg.tile([128, 384], F32, tag="pagg")
                    pairs = [(p, min(p + 2, K0)) for p in range(0, K0, 2)]

                    def stage_a(p0, p1):
                        """e1T + e1 for chunks [p0, p1): produce e1T (bf16) and
                        the e1 halves of the ef tiles."""
                        g2 = slice(p0 * 128, p1 * 128)
                        gw = g2.stop - g2.start
                        pe1T = psB.tile([128, 512], F32, tag="pe1T")
                        nc.tensor.matmul(pe1T[:, 0:gw], We1T[:, 0:128],
                                         eftt[:, g2], start=True, stop=True)
                        nc.tensor.matmul(pe1T[:, 256 : 256 + gw], We1T[:, 128:256],
                                         eftt[:, g2], start=True, stop=True)
                        e1T = esb.tile([128, 512], BF16, tag="e1T")
                        nc.vector.tensor_scalar_max(e1T[:], pe1T[:], 0.0)
                        efs = []
                        for k in range(p0, p1):
                            sl = slice(k * 128, (k + 1) * 128)
                            pe1 = psA.tile([128, 256], F32, tag="pe1")
                            nc.tensor.matmul(pe1[:], eftt[:, sl], We1T[:],
                                             start=True, stop=True)
                            ef = esb.tile([128, 384], BF16, tag="ef")
                            nc.scalar.activation(ef[:, 0:256], pe1[:], Relu)
                            efs.append(ef)
                        return e1T, efs

                    def stage_b(p0, p1, e1T, efs):
                        """e2 + aggregation for chunks [p0, p1), consuming the
                        e1T produced a pair earlier."""
                        for k in range(p0, p1):
                            sl = slice(k * 128, (k + 1) * 128)
                            ko = (k - p0) * 128
                            ef = efs[k - p0]
                            pe2 = psC.tile([128, 128], F32, tag="pe2")
                            nc.tensor.matmul(pe2[:], e1T[:, ko : ko + 128],
                                             We2T[:, 0:128], start=True, stop=False)
                            nc.tensor.matmul(pe2[:], e1T[:, 256 + ko : 256 + ko + 128],
                                             We2T[:, 128:256], start=False, stop=False)
                            nc.tensor.matmul(pe2[:], ght[:, sl], g2aug[:],
                                             start=False, stop=True)
                            nc.vector.tensor_scalar_max(ef[:, 256:384], pe2[:], 0.0)
                            nc.tensor.matmul(pagg[:], ohall[:, sl], ef[:],
                                             start=(k == 0), stop=(k == K0 - 1))

                    prev = None
                    for (p0, p1) in pairs:
                        cur = (p0, p1, *stage_a(p0, p1))
                        if prev is not None:
                            stage_b(*prev)
                        prev = cur
                    stage_b(*prev)

                    nc.scalar.activation(
                        aggall[:, t * 384 : (t + 1) * 384], pagg[:], Copy,
                        scale=invc_t[:],
                    )

            # ----------------- node phase -----------------
            with tc.tile_pool(name="np_", bufs=2) as np_, \
                 tc.tile_pool(name="nsb", bufs=3) as nsb, \
                 tc.tile_pool(name="npsA", bufs=2, space=bass.MemorySpace.PSUM) as npsA, \
                 tc.tile_pool(name="npsB", bufs=2, space=bass.MemorySpace.PSUM) as npsB, \
                 tc.tile_pool(name="npsC", bufs=2, space=bass.MemorySpace.PSUM) as npsC, \
                 tc.tile_pool(name="npsP", bufs=1, space=bass.MemorySpace.PSUM) as npsP:
                ppN = npsP.tile([2, 128], F32, tag="ppN")
                ppE = npsP.tile([2, 128], F32, tag="ppE")
                for t in range(NT):
                    aggsl = aggall[:, t * 384 : (t + 1) * 384]
                    pT = npsA.tile([128, 384], BF16, tag="pT")
                    nc.tensor.transpose(pT[:, 0:128], aggsl[:, 0:128], ident[:])
                    nc.tensor.transpose(pT[:, 128:256], aggsl[:, 128:256], ident[:])
                    nc.tensor.transpose(pT[:, 256:384], aggsl[:, 256:384], ident[:])
                    aggT = nsb.tile([128, 384], BF16, tag="aggT")
                    nc.vector.tensor_copy(aggT[:], pT[:])

                    nftt = np_.tile([65, 128], BF16, tag="nftt")
                    nc.sync.dma_start(nftt[:], d_nft[:, t * 128 : (t + 1) * 128])
                    nht = np_.tile([3, 128], BF16, tag="nht")
                    nc.sync.dma_start(nht[:], d_nhot[:, t * 128 : (t + 1) * 128])
                    pw = np_.tile([128, 4], BF16, tag="pw")
                    nc.sync.dma_start(pw[:], d_poolw[t * 128 : (t + 1) * 128, :])

                    pn1 = npsB.tile([128, 256], F32, tag="pn1")
                    for s in (0, 1):
                        ssl = slice(s * 128, (s + 1) * 128)
                        nc.tensor.matmul(pn1[:, ssl], Wn1T[:, ssl], nftt[:], start=True, stop=False)
                        nc.tensor.matmul(pn1[:, ssl], Win1T[:, s * 128 : s * 128 + 128],
                                         aggT[:, 0:128], start=False, stop=False)
                        nc.tensor.matmul(pn1[:, ssl], Win1T[:, 256 + s * 128 : 256 + s * 128 + 128],
                                         aggT[:, 128:256], start=False, stop=True)
                    n1T = nsb.tile([128, 256], BF16, tag="n1T")
                    nc.scalar.activation(n1T[:], pn1[:], Relu)

                    pn2 = npsC.tile([128, 128], F32, tag="pn2")
                    nc.tensor.matmul(pn2[:], n1T[:, 0:128], Wn2T[:, 0:128], start=True, stop=False)
                    nc.tensor.matmul(pn2[:], n1T[:, 128:256], Wn2T[:, 128:256], start=False, stop=False)
                    nc.tensor.matmul(pn2[:], aggT[:, 256:384], Win2T[:], start=False, stop=False)
                    nc.tensor.matmul(pn2[:], nht[:], gnaug[:], start=False, stop=True)
                    n2 = nsb.tile([128, 128], BF16, tag="n2")
                    nc.scalar.activation(n2[:], pn2[:], Relu)

                    nc.tensor.matmul(ppN[:], pw[:, 0:2], n2[:],
                                     start=(t == 0), stop=(t == NT - 1))
                    nc.tensor.matmul(ppE[:], pw[:, 2:4], aggsl[:, 256:384],
                                     start=(t == 0), stop=(t == NT - 1))

                # ----------------- final projection -----------------
                navg = nsb.tile([2, 128], F32, tag="navg")
                nc.scalar.activation(navg[:], ppN[:], Copy)
                eavg = nsb.tile([2, 128], F32, tag="eavg")
                nc.scalar.activation(eavg[:], ppE[:], Copy)

                ptr2 = npsA.tile([128, 4], F32, tag="pT")
                nc.tensor.transpose(ptr2[:, 0:2], navg[:], ident2[:])
                nc.tensor.transpose(ptr2[:, 2:4], eavg[:], ident2[:])
                nt2 = nsb.tile([128, 4], F32, tag="nt2")
                nc.scalar.activation(nt2[:], ptr2[:], Copy)

                pout = npsC.tile([128, 2], F32, tag="pn2")
                nc.tensor.matmul(pout[:], WgnT[:], nt2[:, 0:2], start=True, stop=False)
                nc.tensor.matmul(pout[:], WgeT[:], nt2[:, 2:4], start=False, stop=False)
                nc.tensor.matmul(pout[:], WggT[:], globT[:], start=False, stop=False)
                nc.tensor.matmul(pout[:], bgr[:], ones2[:], start=False, stop=True)
                outsb = nsb.tile([128, 2], F32, tag="outsb")
                nc.scalar.activation(outsb[:], pout[:], Copy)
                nc.sync.dma_start(d_out[:], outsb[:])

    return nc


_CACHE = {}


def _get_nc(NT, K0):
    key = (NT, K0)
    if key not in _CACHE:
        _CACHE[key] = _build(NT, K0)
    return _CACHE[key]


def _run(inputs, trace=False):
    in_maps, NT, K0 = _prepare(inputs)
    nc = _get_nc(NT, K0)
    res = run_bass_kernel_spmd(nc, in_maps, list(range(N_CORES)), trace=trace)
    out = np.zeros((N_GRAPHS, 128), np.float32)
    for c in range(N_CORES):
        r = np.asarray(res.results[c]["out"], np.float32)
        out[GPC * c] = r[:, 0]
        out[GPC * c + 1] = r[:, 1]
    return out, res


def kernel(**inputs):
    out, _ = _run(inputs, trace=False)
    return out


def kernel_traced(**inputs):
    return _run(inputs, trace=True)



# revision 7
# speedup vs baseline: 2.1590x; 2.1590x over previous
"""Trainium2 Bass kernel for a 2-layer GraphNetwork (gnn_message_passing).

Strategy (v2):
  - 16 graphs partitioned across 8 cores, 2 graphs per core, paired
    big+small by edge count to balance load. All segment reductions are
    core-local; [16,128] output rows are gathered on the host.
  - Per core, nodes are bin-packed (LPT) into NT tiles of 128 slots; each
    tile's incoming edges are padded to K0 chunks of 128. Segment sums run
    on the tensor engine as one-hot matmuls with HOST-built one-hot tiles.
  - Every hot-loop matmul uses a full K=128 stationary: edge/node feature
    tiles are zero-padded to 128 partitions, with ones/graph-indicator
    rows folded in so biases and global-feature terms are matmul
    accumulations against padded weight tiles. (Partial-K matmuls throttle
    the PE clock to 1.2 GHz; full-K keeps it at 2.4 GHz.)
  - bf16 inputs/intermediates, fp32 PSUM accumulation, fp32 final stage.
"""

import numpy as np
import ml_dtypes

import concourse.bass as bass
import concourse.tile as tile_mod
from concourse import tile
from concourse.bass_utils import run_bass_kernel_spmd
from concourse.vector_clock import ScopedClock

mybir = bass.mybir

N_NODES, N_EDGES, N_GRAPHS = 20000, 320000, 16
F_NODE, F_EDGE, F_GLOB = 64, 32, 16
N_CORES = 8
GPC = N_GRAPHS // N_CORES  # graphs per core = 2

BF16 = mybir.dt.bfloat16
F32 = mybir.dt.float32
npbf16 = ml_dtypes.bfloat16

# ---------------------------------------------------------------------------
# Workaround: CoreV3 codegen rejects instructions carrying more than one
# semaphore wait. Split the waits across extra no-ops.
_MAX_WAITS = 1
_ENGINE_WAIT_LIMIT = 1
_SPLIT_ENGINES = None  # set lazily


def _split_excess_waits(nc):
    global _SPLIT_ENGINES
    if _SPLIT_ENGINES is None:
        ET = mybir.EngineType
        _SPLIT_ENGINES = {ET.PE, ET.Activation, ET.DVE, ET.SP, ET.Pool}
    ctr = [0]
    for bass_bb in nc.bb_map.values():
        bb = bass_bb.bb
        il = bb.instructions
        out = []
        changed = False
        for inst in il:
            si = inst.sync_info
            waits = list(si.on_wait) if (si and si.on_wait) else []
            if len(waits) > _ENGINE_WAIT_LIMIT and inst.engine in _SPLIT_ENGINES:
                head, keep = waits[:-_ENGINE_WAIT_LIMIT], waits[-_ENGINE_WAIT_LIMIT:]
                for i in range(0, len(head), _ENGINE_WAIT_LIMIT):
                    nop = mybir.InstNoOp(name=f"waitsplit-{ctr[0]}", ins=[], outs=[])
                    ctr[0] += 1
                    nop.engine = inst.engine
                    nop.sync_info = mybir.SyncInfo(
                        on_wait=head[i : i + _ENGINE_WAIT_LIMIT], on_update=[]
                    )
                    nc.register_instruction(nop, overwrite=True)
                    out.append(nop)
                inst.sync_info = mybir.SyncInfo(
                    on_wait=keep, on_update=list(si.on_update or [])
                )
                changed = True
            out.append(inst)
        if changed:
            bb.instructions = out


def _split_drain_and_barrier(self, tick_clock, wait_clock):
    nc = self.nc
    _split_excess_waits(nc)
    drain_inst = nc.sync.drain()
    wait_clock.add_sem_waits(
        drain_inst.ins, ScopedClock({None: tick_clock.global_clock})
    )
    mi = drain_inst.ins
    waits = list(mi.sync_info.on_wait) if (mi.sync_info and mi.sync_info.on_wait) else []
    if len(waits) > _MAX_WAITS:
        upd = list(mi.sync_info.on_update) if mi.sync_info.on_update else []
        mi.sync_info = mybir.SyncInfo(on_wait=waits[:_MAX_WAITS], on_update=upd)
        for i in range(_MAX_WAITS, len(waits), _MAX_WAITS):
            nop = nc.sync.nop(nofuse=True)
            nop.ins.sync_info = mybir.SyncInfo(
                on_wait=waits[i : i + _MAX_WAITS], on_update=[]
            )
    nc.all_engine_barrier()
    assert self.sems is not None
    popped = nc._tile_sem_poison_stack.pop()
    assert popped is self._sem_poison
    nc.clear_and_free_semaphores(list(self.sems.allocated().values()))
    nc.all_engine_barrier()


tile_mod.TileContext._drain_and_barrier = _split_drain_and_barrier


# ---------------------------------------------------------------------------
# Host-side graph partitioning / layout


def _pack_core(node_ids, degs, nt, cap_e):
    """LPT: place nodes (descending degree) onto the least-edge-loaded tile
    that still has node capacity. Returns per-tile node-id arrays, or None
    if some tile exceeds cap_e edges."""
    order = np.argsort(-degs, kind="stable")
    tiles_n = [[] for _ in range(nt)]
    tile_ncnt = np.zeros(nt, np.int64)
    tile_ecnt = np.zeros(nt, np.int64)
    for j in order:
        cand = np.where(tile_ncnt < 128)[0]
        if len(cand) == 0:
            return None
        t = cand[np.argmin(tile_ecnt[cand])]
        tiles_n[t].append(node_ids[j])
        tile_ncnt[t] += 1
        tile_ecnt[t] += degs[j]
    if (tile_ecnt > cap_e).any():
        return None
    return [np.array(t, dtype=np.int64) for t in tiles_n]


def _prepare(inputs):
    nf = np.asarray(inputs["node_feats"], np.float32)
    ef = np.asarray(inputs["edge_feats"], np.float32)
    glob = np.asarray(inputs["globals_"], np.float32)
    recv = np.asarray(inputs["receivers"]).astype(np.int64)
    ngraph = np.asarray(inputs["node_graph"]).astype(np.int64)

    cnt = np.bincount(recv, minlength=N_NODES).astype(np.int64)
    egraph = ngraph[recv]
    ncnt_g = np.bincount(ngraph, minlength=N_GRAPHS)
    ecnt_g = np.bincount(egraph, minlength=N_GRAPHS)

    # pair graphs big+small by edge count to balance cores
    order = np.argsort(-ecnt_g, kind="stable")
    pairs = [(int(order[i]), int(order[N_GRAPHS - 1 - i])) for i in range(N_CORES)]

    core_nodes = [
        np.where((ngraph == pa) | (ngraph == pb))[0] for pa, pb in pairs
    ]
    NT = int(max((len(cn) + 127) // 128 for cn in core_nodes))

    core_of_graph = np.zeros(N_GRAPHS, np.int64)
    for c, (pa, pb) in enumerate(pairs):
        core_of_graph[pa] = c
        core_of_graph[pb] = c
    edge_core = core_of_graph[egraph]
    ecnt_core = np.bincount(edge_core, minlength=N_CORES)

    packs = None
    K0 = max(1, int(max(ecnt_core) + NT * 128 - 1) // (NT * 128))
    K0 = ((K0 + 3) // 4) * 4  # quad-chunk device loops need K0 % 4 == 0
    for k0 in range(K0, K0 + 12, 4):
        trial = []
        ok = True
        for c in range(N_CORES):
            p = _pack_core(core_nodes[c], cnt[core_nodes[c]], NT, k0 * 128)
            if p is None:
                ok = False
                break
            trial.append(p)
        if ok:
            packs, K0 = trial, k0
            break
    assert packs is not None, "bin packing failed"

    NPAD = NT * 128
    EPAD = NT * K0 * 128

    # replicated weights
    w_np = {}

    def bf(x):
        return np.ascontiguousarray(x).astype(npbf16)

    We1T = np.zeros((128, 256), np.float32)
    We1T[:32] = np.asarray(inputs["We1"], np.float32).T
    We1T[32] = np.asarray(inputs["be1"], np.float32)
    w_np["We1T"] = bf(We1T)

    We2T = np.asarray(inputs["We2"], np.float32).T  # [256, 128]
    w_np["We2T"] = bf(np.concatenate([We2T[:128], We2T[128:]], axis=1))  # [128,256]

    Wn1T = np.zeros((128, 256), np.float32)
    Wn1T[:64] = np.asarray(inputs["Wn1"], np.float32).T
    Wn1T[64] = np.asarray(inputs["bn1"], np.float32)
    w_np["Wn1T"] = bf(Wn1T)

    Win1T = np.asarray(inputs["Win1"], np.float32).T  # [256 fi, 256 fo]
    w_np["Win1T"] = bf(
        np.concatenate(
            [Win1T[:128, :128], Win1T[:128, 128:], Win1T[128:, :128], Win1T[128:, 128:]],
            axis=1,
        )
    )  # [128, 512]

    Wn2T = np.asarray(inputs["Wn2"], np.float32).T  # [256, 128]
    w_np["Wn2T"] = bf(np.concatenate([Wn2T[:128], Wn2T[128:]], axis=1))  # [128,256]
    w_np["Win2T"] = bf(np.asarray(inputs["Win2"], np.float32).T)  # [128, 128]

    WggT_pad = np.zeros((128, 128), np.float32)
    WggT_pad[:16] = np.asarray(inputs["Wgg"], np.float32).T
    w_np["WgnT"] = np.asarray(inputs["Wgn"], np.float32).T.copy()
    w_np["WgeT"] = np.asarray(inputs["Wge"], np.float32).T.copy()
    w_np["WggT"] = WggT_pad
    bgr_pad = np.zeros((128, 128), np.float32)
    bgr_pad[0] = np.asarray(inputs["bg"], np.float32)
    w_np["bgr"] = bgr_pad
    onesc = np.zeros((128, 2), np.float32)
    onesc[0] = 1.0
    w_np["onesc"] = onesc
    w_np["identb"] = np.eye(128, dtype=npbf16)
    w_np["ident4"] = np.eye(4, dtype=np.float32)

    Wg2 = np.asarray(inputs["Wg2"], np.float32)  # [128, 16]
    Wng2 = np.asarray(inputs["Wng2"], np.float32)
    be2 = np.asarray(inputs["be2"], np.float32)
    bn2 = np.asarray(inputs["bn2"], np.float32)

    # per-core inputs
    in_maps = []
    slot_of_node = np.full(N_NODES, -1, np.int64)
    tile_of_node = np.full(N_NODES, -1, np.int64)
    for c in range(N_CORES):
        pa, pb = pairs[c]
        for t in range(NT):
            ids = packs[c][t]
            slot_of_node[ids] = t * 128 + np.arange(len(ids))
            tile_of_node[ids] = t

        # ---- edges
        eidx = np.where(edge_core == c)[0]
        et = tile_of_node[recv[eidx]]
        order_e = np.argsort(et, kind="stable")
        eidx = eidx[order_e]
        et = et[order_e]
        counts = np.bincount(et, minlength=NT)
        starts = np.concatenate([[0], np.cumsum(counts)[:-1]])
        off_in = np.arange(len(eidx)) - np.repeat(starts, counts)
        dst = et * (K0 * 128) + off_in
        assert (counts <= K0 * 128).all()

        eftT = np.zeros((128, EPAD), np.float32)
        eftT[:32, dst] = ef[eidx].T
        eftT[32, dst] = 1.0
        eg = egraph[eidx]
        eftT[33, dst] = (eg == pa)
        eftT[34, dst] = (eg == pb)

        # one-hot: per 128-edge chunk a [lane, slot] block, K(partitions)=lanes
        onehot = np.zeros((128, EPAD), np.float32)
        onehot[dst % 128, (dst // 128) * 128 + slot_of_node[recv[eidx]] % 128] = 1.0

        # ---- nodes
        slot_node = np.full(NPAD, -1, np.int64)
        for t in range(NT):
            ids = packs[c][t]
            slot_node[t * 128 : t * 128 + len(ids)] = ids
        valid = slot_node >= 0
        sn = np.where(valid, slot_node, 0)

        nftT = np.zeros((128, NPAD), np.float32)
        nftT[:64, valid] = nf[sn[valid]].T
        nftT[64, valid] = 1.0
        ng = ngraph[sn]
        nftT[65] = valid * (ng == pa)
        nftT[66] = valid * (ng == pb)

        invc = np.ones((NPAD, 1), np.float32)
        invc[valid, 0] = 1.0 / np.maximum(cnt[sn[valid]], 1)

        poolw = np.zeros((NPAD, 128), np.float32)
        for g, gid in enumerate((pa, pb)):
            m = valid & (ng == gid)
            poolw[m, g] = 1.0 / max(ncnt_g[gid], 1)
            poolw[m, 2 + g] = cnt[sn[m]] / max(ecnt_g[gid], 1)

        g2aug = np.zeros((128, 128), np.float32)
        g2aug[32] = be2
        g2aug[33] = Wg2 @ glob[pa]
        g2aug[34] = Wg2 @ glob[pb]

        gnaug = np.zeros((128, 128), np.float32)
        gnaug[64] = bn2
        gnaug[65] = Wng2 @ glob[pa]
        gnaug[66] = Wng2 @ glob[pb]

        globT = np.zeros((128, 2), np.float32)
        globT[:16, 0] = glob[pa]
        globT[:16, 1] = glob[pb]

        m = {
            "eft": bf(eftT),
            "onehot": bf(onehot),
            "nft": bf(nftT),
            "invc": invc,
            "poolw": bf(poolw),
            "g2aug": bf(g2aug),
            "gnaug": bf(gnaug),
            "globT": globT,
        }
        m.update(w_np)
        in_maps.append(m)

    return in_maps, NT, K0, pairs


# ---------------------------------------------------------------------------
# Device program (identical on all cores)


def _build(NT, K0):
    nc = bass.Bass()
    NPAD = NT * 128
    EPAD = NT * K0 * 128
    CW = K0 * 128  # edge columns per node-tile

    d_eft = nc.dram_tensor("eft", [128, EPAD], BF16, kind="ExternalInput")
    d_onehot = nc.dram_tensor("onehot", [128, EPAD], BF16, kind="ExternalInput")
    d_nft = nc.dram_tensor("nft", [128, NPAD], BF16, kind="ExternalInput")
    d_invc = nc.dram_tensor("invc", [NPAD, 1], F32, kind="ExternalInput")
    d_poolw = nc.dram_tensor("poolw", [NPAD, 128], BF16, kind="ExternalInput")
    d_g2aug = nc.dram_tensor("g2aug", [128, 128], BF16, kind="ExternalInput")
    d_gnaug = nc.dram_tensor("gnaug", [128, 128], BF16, kind="ExternalInput")
    d_globT = nc.dram_tensor("globT", [128, 2], F32, kind="ExternalInput")

    d_We1T = nc.dram_tensor("We1T", [128, 256], BF16, kind="ExternalInput")
    d_We2T = nc.dram_tensor("We2T", [128, 256], BF16, kind="ExternalInput")
    d_Wn1T = nc.dram_tensor("Wn1T", [128, 256], BF16, kind="ExternalInput")
    d_Win1T = nc.dram_tensor("Win1T", [128, 512], BF16, kind="ExternalInput")
    d_Wn2T = nc.dram_tensor("Wn2T", [128, 256], BF16, kind="ExternalInput")
    d_Win2T = nc.dram_tensor("Win2T", [128, 128], BF16, kind="ExternalInput")
    d_WgnT = nc.dram_tensor("WgnT", [128, 128], F32, kind="ExternalInput")
    d_WgeT = nc.dram_tensor("WgeT", [128, 128], F32, kind="ExternalInput")
    d_WggT = nc.dram_tensor("WggT", [128, 128], F32, kind="ExternalInput")
    d_bgr = nc.dram_tensor("bgr", [128, 128], F32, kind="ExternalInput")
    d_onesc = nc.dram_tensor("onesc", [128, 2], F32, kind="ExternalInput")
    d_identb = nc.dram_tensor("identb", [128, 128], BF16, kind="ExternalInput")
    d_ident4 = nc.dram_tensor("ident4", [4, 4], F32, kind="ExternalInput")

    d_out = nc.dram_tensor("out", [128, 2], F32, kind="ExternalOutput")

    Relu = mybir.ActivationFunctionType.Relu
    Copy = mybir.ActivationFunctionType.Copy
    NQ = K0 // 2   # e1 chunk-pairs per tile
    NR = K0 // 4   # e2 quads per tile
    NS = K0 // 2   # e1T 256-edge slabs per tile

    with tile.TileContext(nc) as tc:
        with tc.tile_pool(name="wp", bufs=1) as wp:
            def wtile(dram, shape, dt):
                t = wp.tile(shape, dt, tag=dram.name)
                nc.sync.dma_start(t[:], dram[:])
                return t

            We1T = wtile(d_We1T, [128, 256], BF16)
            We2T = wtile(d_We2T, [128, 256], BF16)
            Wn1T = wtile(d_Wn1T, [128, 256], BF16)
            Win1T = wtile(d_Win1T, [128, 512], BF16)
            Wn2T = wtile(d_Wn2T, [128, 256], BF16)
            Win2T = wtile(d_Win2T, [128, 128], BF16)
            g2aug = wtile(d_g2aug, [128, 128], BF16)
            gnaug = wtile(d_gnaug, [128, 128], BF16)
            WgnT = wtile(d_WgnT, [128, 128], F32)
            WgeT = wtile(d_WgeT, [128, 128], F32)
            WggT = wtile(d_WggT, [128, 128], F32)
            bgr = wtile(d_bgr, [128, 128], F32)
            onesc = wtile(d_onesc, [128, 2], F32)
            identb = wtile(d_identb, [128, 128], BF16)
            ident4 = wtile(d_ident4, [4, 4], F32)
            globT = wtile(d_globT, [128, 2], F32)

            aggall = wp.tile([128, 384 * NT], BF16, tag="aggall")

            # ----------------- edge phase -----------------
            with tc.tile_pool(name="ep", bufs=3) as ep, \
                 tc.tile_pool(name="e1p", bufs=2) as e1p, \
                 tc.tile_pool(name="efp", bufs=2 * NQ + 2) as efp, \
                 tc.tile_pool(name="e2p", bufs=NR + 2) as e2p, \
                 tc.tile_pool(name="psA", bufs=2, space=bass.MemorySpace.PSUM) as psA, \
                 tc.tile_pool(name="psB", bufs=2, space=bass.MemorySpace.PSUM) as psB, \
                 tc.tile_pool(name="psC", bufs=2, space=bass.MemorySpace.PSUM) as psC, \
                 tc.tile_pool(name="psAgg", bufs=1, space=bass.MemorySpace.PSUM) as psAgg:
                for t in range(NT):
                    eftt = ep.tile([128, CW], BF16, tag="eftt", bufs=2)
                    nc.sync.dma_start(eftt[:], d_eft[:, t * CW : (t + 1) * CW])
                    oht = ep.tile([128, CW], BF16, tag="oht", bufs=2)
                    nc.sync.dma_start(oht[:], d_onehot[:, t * CW : (t + 1) * CW])
                    invc_t = ep.tile([128, 1], F32, tag="invc")
                    nc.sync.dma_start(invc_t[:], d_invc[t * 128 : (t + 1) * 128, :])

                    # e1T: feat-major halves, 256-edge slabs
                    e1h0 = e1p.tile([128, CW], BF16, tag="e1h0")
                    e1h1 = e1p.tile([128, CW], BF16, tag="e1h1")
                    for s in range(NS):
                        sl = slice(s * 256, (s + 1) * 256)
                        pT = psA.tile([128, 512], F32, tag="pT")
                        nc.tensor.matmul(pT[:, 0:256], We1T[:, 0:128],
                                         eftt[:, sl], start=True, stop=True)
                        nc.tensor.matmul(pT[:, 256:512], We1T[:, 128:256],
                                         eftt[:, sl], start=True, stop=True)
                        nc.scalar.activation(e1h0[:, sl], pT[:, 0:256], Relu)
                        nc.vector.tensor_scalar_max(e1h1[:, sl], pT[:, 256:512], 0.0)

                    # e1 edge-major, 2-chunk pairs
                    ef1s = []
                    for q in range(NQ):
                        pE = psB.tile([128, 512], F32, tag="pE")
                        for j in (0, 1):
                            c = 2 * q + j
                            nc.tensor.matmul(pE[:, j * 256 : (j + 1) * 256],
                                             eftt[:, c * 128 : (c + 1) * 128],
                                             We1T[:], start=True, stop=True)
                        ef1 = efp.tile([128, 512], BF16, tag="ef1")
                        if q % 2 == 0:
                            nc.scalar.activation(ef1[:], pE[:], Relu)
                        else:
                            nc.vector.tensor_scalar_max(ef1[:], pE[:], 0.0)
                        ef1s.append(ef1)

                    # e2 edge-major, 4-chunk quads
                    e2s = []
                    for r in range(NR):
                        pC = psC.tile([128, 512], F32, tag="pC")
                        for j in range(4):
                            c = 4 * r + j
                            csl = slice(c * 128, (c + 1) * 128)
                            osl = slice(j * 128, (j + 1) * 128)
                            nc.tensor.matmul(pC[:, osl], e1h0[:, csl],
                                             We2T[:, 0:128], start=True, stop=False)
                            nc.tensor.matmul(pC[:, osl], e1h1[:, csl],
                                             We2T[:, 128:256], start=False, stop=False)
                            nc.tensor.matmul(pC[:, osl], eftt[:, csl],
                                             g2aug[:], start=False, stop=True)
                        e2sb = e2p.tile([128, 512], BF16, tag="e2sb")
                        nc.vector.tensor_scalar_max(e2sb[:], pC[:], 0.0)
                        e2s.append(e2sb)

                    # aggregation
                    pagg1 = psAgg.tile([128, 256], F32, tag="pagg1")
                    pagg2 = psAgg.tile([128, 128], F32, tag="pagg2")
                    for c in range(K0):
                        csl = slice(c * 128, (c + 1) * 128)
                        nc.tensor.matmul(pagg1[:], oht[:, csl],
                                         ef1s[c // 2][:, (c % 2) * 256 : (c % 2) * 256 + 256],
                                         start=(c == 0), stop=(c == K0 - 1))
                        nc.tensor.matmul(pagg2[:], oht[:, csl],
                                         e2s[c // 4][:, (c % 4) * 128 : (c % 4) * 128 + 128],
                                         start=(c == 0), stop=(c == K0 - 1))

                    nc.scalar.activation(
                        aggall[:, t * 384 : t * 384 + 256], pagg1[:], Copy,
                        scale=invc_t[:],
                    )
                    nc.scalar.activation(
                        aggall[:, t * 384 + 256 : (t + 1) * 384], pagg2[:], Copy,
                        scale=invc_t[:],
                    )

            # ----------------- node phase -----------------
            with tc.tile_pool(name="np_", bufs=2) as np_, \
                 tc.tile_pool(name="nsb", bufs=3) as nsb, \
                 tc.tile_pool(name="npsA", bufs=2, space=bass.MemorySpace.PSUM) as npsA, \
                 tc.tile_pool(name="npsB", bufs=2, space=bass.MemorySpace.PSUM) as npsB, \
                 tc.tile_pool(name="npsC", bufs=1, space=bass.MemorySpace.PSUM) as npsC, \
                 tc.tile_pool(name="npsP", bufs=1, space=bass.MemorySpace.PSUM) as npsP:
                ppN = npsP.tile([128, 128], F32, tag="ppN")
                ppE = npsP.tile([128, 128], F32, tag="ppE")
                for t in range(NT):
                    aggsl = aggall[:, t * 384 : (t + 1) * 384]
                    pT = npsA.tile([128, 384], BF16, tag="pT", bufs=1)
                    nc.tensor.transpose(pT[:, 0:128], aggsl[:, 0:128], identb[:])
                    nc.tensor.transpose(pT[:, 128:256], aggsl[:, 128:256], identb[:])
                    nc.tensor.transpose(pT[:, 256:384], aggsl[:, 256:384], identb[:])
                    aggT = nsb.tile([128, 384], BF16, tag="aggT")
                    nc.vector.tensor_copy(aggT[:], pT[:])

                    nftt = np_.tile([128, 128], BF16, tag="nftt")
                    nc.sync.dma_start(nftt[:], d_nft[:, t * 128 : (t + 1) * 128])
                    pw = np_.tile([128, 128], BF16, tag="pw")
                    nc.sync.dma_start(pw[:], d_poolw[t * 128 : (t + 1) * 128, :])

                    pn1 = npsB.tile([128, 256], F32, tag="pn1")
                    for s in (0, 1):
                        ssl = slice(s * 128, (s + 1) * 128)
                        nc.tensor.matmul(pn1[:, ssl], Wn1T[:, ssl], nftt[:],
                                         start=True, stop=False)
                        nc.tensor.matmul(pn1[:, ssl], Win1T[:, s * 128 : s * 128 + 128],
                                         aggT[:, 0:128], start=False, stop=False)
                        nc.tensor.matmul(pn1[:, ssl], Win1T[:, 256 + s * 128 : 256 + s * 128 + 128],
                                         aggT[:, 128:256], start=False, stop=True)
                    n1T = nsb.tile([128, 256], BF16, tag="n1T")
                    nc.scalar.activation(n1T[:], pn1[:], Relu)

                    pn2 = npsC.tile([128, 128], F32, tag="pn2")
                    nc.tensor.matmul(pn2[:], n1T[:, 0:128], Wn2T[:, 0:128], start=True, stop=False)
                    nc.tensor.matmul(pn2[:], n1T[:, 128:256], Wn2T[:, 128:256], start=False, stop=False)
                    nc.tensor.matmul(pn2[:], aggT[:, 256:384], Win2T[:], start=False, stop=False)
                    nc.tensor.matmul(pn2[:], nftt[:], gnaug[:], start=False, stop=True)
                    n2 = nsb.tile([128, 128], BF16, tag="n2")
                    nc.vector.tensor_scalar_max(n2[:], pn2[:], 0.0)

                    nc.tensor.matmul(ppN[:], pw[:], n2[:],
                                     start=(t == 0), stop=(t == NT - 1))
                    nc.tensor.matmul(ppE[:], pw[:], aggsl[:, 256:384],
                                     start=(t == 0), stop=(t == NT - 1))

                # ----------------- final projection -----------------
                pp4 = nsb.tile([4, 256], F32, tag="pp4")
                nc.scalar.activation(pp4[:, 0:128], ppN[0:4, :], Copy)
                nc.scalar.activation(pp4[:, 128:256], ppE[0:4, :], Copy)

                ptr8 = npsP.tile([128, 8], F32, tag="ptr8")
                nc.tensor.transpose(ptr8[:, 0:4], pp4[:, 0:128], ident4[:])
                nc.tensor.transpose(ptr8[:, 4:8], pp4[:, 128:256], ident4[:])
                nt8 = nsb.tile([128, 8], F32, tag="nt8")
                nc.scalar.activation(nt8[:], ptr8[:], Copy)

                pout = npsP.tile([128, 2], F32, tag="pout")
                nc.tensor.matmul(pout[:], WgnT[:], nt8[:, 0:2], start=True, stop=False)
                nc.tensor.matmul(pout[:], WgeT[:], nt8[:, 6:8], start=False, stop=False)
                nc.tensor.matmul(pout[:], WggT[:], globT[:], start=False, stop=False)
                nc.tensor.matmul(pout[:], bgr[:], onesc[:], start=False, stop=True)
                outsb = nsb.tile([128, 2], F32, tag="outsb")
                nc.scalar.activation(outsb[:], pout[:], Copy)
                nc.sync.dma_start(d_out[:], outsb[:])

    return nc


_CACHE = {}


def _get_nc(NT, K0):
    key = (NT, K0)
    if key not in _CACHE:
        _CACHE[key] = _build(NT, K0)
    return _CACHE[key]


def _run(inputs, trace=False):
    in_maps, NT, K0, pairs = _prepare(inputs)
    nc = _get_nc(NT, K0)
    res = run_bass_kernel_spmd(nc, in_maps, list(range(N_CORES)), trace=trace)
    out = np.zeros((N_GRAPHS, 128), np.float32)
    for c in range(N_CORES):
        r = np.asarray(res.results[c]["out"], np.float32)
        pa, pb = pairs[c]
        out[pa] = r[:, 0]
        out[pb] = r[:, 1]
    return out, res


def kernel(**inputs):
    out, _ = _run(inputs, trace=False)
    return out


def kernel_traced(**inputs):
    return _run(inputs, trace=True)


# revision 8
# speedup vs baseline: 2.2018x; 1.0198x over previous
"""Trainium2 Bass kernel for a 2-layer GraphNetwork (gnn_message_passing).

Strategy (v2):
  - 16 graphs partitioned across 8 cores, 2 graphs per core, paired
    big+small by edge count to balance load. All segment reductions are
    core-local; [16,128] output rows are gathered on the host.
  - Per core, nodes are bin-packed (LPT) into NT tiles of 128 slots; each
    tile's incoming edges are padded to K0 chunks of 128. Segment sums run
    on the tensor engine as one-hot matmuls with HOST-built one-hot tiles.
  - Every hot-loop matmul uses a full K=128 stationary: edge/node feature
    tiles are zero-padded to 128 partitions, with ones/graph-indicator
    rows folded in so biases and global-feature terms are matmul
    accumulations against padded weight tiles. (Partial-K matmuls throttle
    the PE clock to 1.2 GHz; full-K keeps it at 2.4 GHz.)
  - bf16 inputs/intermediates, fp32 PSUM accumulation, fp32 final stage.
"""

import numpy as np
import ml_dtypes

import concourse.bass as bass
import concourse.tile as tile_mod
from concourse import tile
from concourse.bass_utils import run_bass_kernel_spmd
from concourse.vector_clock import ScopedClock

mybir = bass.mybir

N_NODES, N_EDGES, N_GRAPHS = 20000, 320000, 16
F_NODE, F_EDGE, F_GLOB = 64, 32, 16
N_CORES = 8
GPC = N_GRAPHS // N_CORES  # graphs per core = 2

BF16 = mybir.dt.bfloat16
F32 = mybir.dt.float32
npbf16 = ml_dtypes.bfloat16

# ---------------------------------------------------------------------------
# Workaround: CoreV3 codegen rejects instructions carrying more than one
# semaphore wait. Split the waits across extra no-ops.
_MAX_WAITS = 1
_ENGINE_WAIT_LIMIT = 1
_SPLIT_ENGINES = None  # set lazily


def _split_excess_waits(nc):
    global _SPLIT_ENGINES
    if _SPLIT_ENGINES is None:
        ET = mybir.EngineType
        _SPLIT_ENGINES = {ET.PE, ET.Activation, ET.DVE, ET.SP, ET.Pool}
    ctr = [0]
    for bass_bb in nc.bb_map.values():
        bb = bass_bb.bb
        il = bb.instructions
        out = []
        changed = False
        for inst in il:
            si = inst.sync_info
            waits = list(si.on_wait) if (si and si.on_wait) else []
            if len(waits) > _ENGINE_WAIT_LIMIT and inst.engine in _SPLIT_ENGINES:
                head, keep = waits[:-_ENGINE_WAIT_LIMIT], waits[-_ENGINE_WAIT_LIMIT:]
                for i in range(0, len(head), _ENGINE_WAIT_LIMIT):
                    nop = mybir.InstNoOp(name=f"waitsplit-{ctr[0]}", ins=[], outs=[])
                    ctr[0] += 1
                    nop.engine = inst.engine
                    nop.sync_info = mybir.SyncInfo(
                        on_wait=head[i : i + _ENGINE_WAIT_LIMIT], on_update=[]
                    )
                    nc.register_instruction(nop, overwrite=True)
                    out.append(nop)
                inst.sync_info = mybir.SyncInfo(
                    on_wait=keep, on_update=list(si.on_update or [])
                )
                changed = True
            out.append(inst)
        if changed:
            bb.instructions = out


def _split_drain_and_barrier(self, tick_clock, wait_clock):
    nc = self.nc
    _split_excess_waits(nc)
    drain_inst = nc.sync.drain()
    wait_clock.add_sem_waits(
        drain_inst.ins, ScopedClock({None: tick_clock.global_clock})
    )
    mi = drain_inst.ins
    waits = list(mi.sync_info.on_wait) if (mi.sync_info and mi.sync_info.on_wait) else []
    if len(waits) > _MAX_WAITS:
        upd = list(mi.sync_info.on_update) if mi.sync_info.on_update else []
        mi.sync_info = mybir.SyncInfo(on_wait=waits[:_MAX_WAITS], on_update=upd)
        for i in range(_MAX_WAITS, len(waits), _MAX_WAITS):
            nop = nc.sync.nop(nofuse=True)
            nop.ins.sync_info = mybir.SyncInfo(
                on_wait=waits[i : i + _MAX_WAITS], on_update=[]
            )
    nc.all_engine_barrier()
    assert self.sems is not None
    popped = nc._tile_sem_poison_stack.pop()
    assert popped is self._sem_poison
    nc.clear_and_free_semaphores(list(self.sems.allocated().values()))
    nc.all_engine_barrier()


tile_mod.TileContext._drain_and_barrier = _split_drain_and_barrier


# ---------------------------------------------------------------------------
# Host-side graph partitioning / layout


def _pack_core(node_ids, degs, nt, cap_e):
    """LPT: place nodes (descending degree) onto the least-edge-loaded tile
    that still has node capacity. Returns per-tile node-id arrays, or None
    if some tile exceeds cap_e edges."""
    order = np.argsort(-degs, kind="stable")
    tiles_n = [[] for _ in range(nt)]
    tile_ncnt = np.zeros(nt, np.int64)
    tile_ecnt = np.zeros(nt, np.int64)
    for j in order:
        cand = np.where(tile_ncnt < 128)[0]
        if len(cand) == 0:
            return None
        t = cand[np.argmin(tile_ecnt[cand])]
        tiles_n[t].append(node_ids[j])
        tile_ncnt[t] += 1
        tile_ecnt[t] += degs[j]
    if (tile_ecnt > cap_e).any():
        return None
    return [np.array(t, dtype=np.int64) for t in tiles_n]


def _prepare(inputs):
    nf = np.asarray(inputs["node_feats"], np.float32)
    ef = np.asarray(inputs["edge_feats"], np.float32)
    glob = np.asarray(inputs["globals_"], np.float32)
    recv = np.asarray(inputs["receivers"]).astype(np.int64)
    ngraph = np.asarray(inputs["node_graph"]).astype(np.int64)

    cnt = np.bincount(recv, minlength=N_NODES).astype(np.int64)
    egraph = ngraph[recv]
    ncnt_g = np.bincount(ngraph, minlength=N_GRAPHS)
    ecnt_g = np.bincount(egraph, minlength=N_GRAPHS)

    # pair graphs big+small by edge count to balance cores
    order = np.argsort(-ecnt_g, kind="stable")
    pairs = [(int(order[i]), int(order[N_GRAPHS - 1 - i])) for i in range(N_CORES)]

    core_nodes = [
        np.where((ngraph == pa) | (ngraph == pb))[0] for pa, pb in pairs
    ]
    NT = int(max((len(cn) + 127) // 128 for cn in core_nodes))

    core_of_graph = np.zeros(N_GRAPHS, np.int64)
    for c, (pa, pb) in enumerate(pairs):
        core_of_graph[pa] = c
        core_of_graph[pb] = c
    edge_core = core_of_graph[egraph]
    ecnt_core = np.bincount(edge_core, minlength=N_CORES)

    packs = None
    K0 = max(1, int(max(ecnt_core) + NT * 128 - 1) // (NT * 128))
    K0 = ((K0 + 3) // 4) * 4  # quad-chunk device loops need K0 % 4 == 0
    for k0 in range(K0, K0 + 12, 4):
        trial = []
        ok = True
        for c in range(N_CORES):
            p = _pack_core(core_nodes[c], cnt[core_nodes[c]], NT, k0 * 128)
            if p is None:
                ok = False
                break
            trial.append(p)
        if ok:
            packs, K0 = trial, k0
            break
    assert packs is not None, "bin packing failed"

    NPAD = NT * 128
    EPAD = NT * K0 * 128

    # replicated weights
    w_np = {}

    def bf(x):
        return np.ascontiguousarray(x).astype(npbf16)

    We1T = np.zeros((128, 256), np.float32)
    We1T[:32] = np.asarray(inputs["We1"], np.float32).T
    We1T[32] = np.asarray(inputs["be1"], np.float32)
    w_np["We1T"] = bf(We1T)

    We2T = np.asarray(inputs["We2"], np.float32).T  # [256, 128]
    w_np["We2T"] = bf(np.concatenate([We2T[:128], We2T[128:]], axis=1))  # [128,256]

    Wn1T = np.zeros((128, 256), np.float32)
    Wn1T[:64] = np.asarray(inputs["Wn1"], np.float32).T
    Wn1T[64] = np.asarray(inputs["bn1"], np.float32)
    w_np["Wn1T"] = bf(Wn1T)

    Win1T = np.asarray(inputs["Win1"], np.float32).T  # [256 fi, 256 fo]
    w_np["Win1T"] = bf(
        np.concatenate(
            [Win1T[:128, :128], Win1T[:128, 128:], Win1T[128:, :128], Win1T[128:, 128:]],
            axis=1,
        )
    )  # [128, 512]

    Wn2T = np.asarray(inputs["Wn2"], np.float32).T  # [256, 128]
    w_np["Wn2T"] = bf(np.concatenate([Wn2T[:128], Wn2T[128:]], axis=1))  # [128,256]
    w_np["Win2T"] = bf(np.asarray(inputs["Win2"], np.float32).T)  # [128, 128]

    WggT_pad = np.zeros((128, 128), np.float32)
    WggT_pad[:16] = np.asarray(inputs["Wgg"], np.float32).T
    w_np["WgnT"] = np.asarray(inputs["Wgn"], np.float32).T.copy()
    w_np["WgeT"] = np.asarray(inputs["Wge"], np.float32).T.copy()
    w_np["WggT"] = WggT_pad
    bgr_pad = np.zeros((128, 128), np.float32)
    bgr_pad[0] = np.asarray(inputs["bg"], np.float32)
    w_np["bgr"] = bgr_pad
    onesc = np.zeros((128, 2), np.float32)
    onesc[0] = 1.0
    w_np["onesc"] = onesc
    w_np["identb"] = np.eye(128, dtype=npbf16)
    w_np["ident4"] = np.eye(4, dtype=np.float32)

    Wg2 = np.asarray(inputs["Wg2"], np.float32)  # [128, 16]
    Wng2 = np.asarray(inputs["Wng2"], np.float32)
    be2 = np.asarray(inputs["be2"], np.float32)
    bn2 = np.asarray(inputs["bn2"], np.float32)

    # per-core inputs
    in_maps = []
    slot_of_node = np.full(N_NODES, -1, np.int64)
    tile_of_node = np.full(N_NODES, -1, np.int64)
    for c in range(N_CORES):
        pa, pb = pairs[c]
        for t in range(NT):
            ids = packs[c][t]
            slot_of_node[ids] = t * 128 + np.arange(len(ids))
            tile_of_node[ids] = t

        # ---- edges
        eidx = np.where(edge_core == c)[0]
        et = tile_of_node[recv[eidx]]
        order_e = np.argsort(et, kind="stable")
        eidx = eidx[order_e]
        et = et[order_e]
        counts = np.bincount(et, minlength=NT)
        starts = np.concatenate([[0], np.cumsum(counts)[:-1]])
        off_in = np.arange(len(eidx)) - np.repeat(starts, counts)
        dst = et * (K0 * 128) + off_in
        assert (counts <= K0 * 128).all()

        eftT = np.zeros((128, EPAD), np.float32)
        eftT[:32, dst] = ef[eidx].T
        eftT[32, dst] = 1.0
        eg = egraph[eidx]
        eftT[33, dst] = (eg == pa)
        eftT[34, dst] = (eg == pb)

        # one-hot: per 128-edge chunk a [lane, slot] block, K(partitions)=lanes
        onehot = np.zeros((128, EPAD), np.float32)
        onehot[dst % 128, (dst // 128) * 128 + slot_of_node[recv[eidx]] % 128] = 1.0

        # ---- nodes
        slot_node = np.full(NPAD, -1, np.int64)
        for t in range(NT):
            ids = packs[c][t]
            slot_node[t * 128 : t * 128 + len(ids)] = ids
        valid = slot_node >= 0
        sn = np.where(valid, slot_node, 0)

        nftT = np.zeros((128, NPAD), np.float32)
        nftT[:64, valid] = nf[sn[valid]].T
        nftT[64, valid] = 1.0
        ng = ngraph[sn]
        nftT[65] = valid * (ng == pa)
        nftT[66] = valid * (ng == pb)

        invc = np.ones((NPAD, 1), np.float32)
        invc[valid, 0] = 1.0 / np.maximum(cnt[sn[valid]], 1)

        poolw = np.zeros((NPAD, 128), np.float32)
        for g, gid in enumerate((pa, pb)):
            m = valid & (ng == gid)
            poolw[m, g] = 1.0 / max(ncnt_g[gid], 1)
            poolw[m, 2 + g] = cnt[sn[m]] / max(ecnt_g[gid], 1)

        g2aug = np.zeros((128, 128), np.float32)
        g2aug[32] = be2
        g2aug[33] = Wg2 @ glob[pa]
        g2aug[34] = Wg2 @ glob[pb]

        gnaug = np.zeros((128, 128), np.float32)
        gnaug[64] = bn2
        gnaug[65] = Wng2 @ glob[pa]
        gnaug[66] = Wng2 @ glob[pb]

        globT = np.zeros((128, 2), np.float32)
        globT[:16, 0] = glob[pa]
        globT[:16, 1] = glob[pb]

        m = {
            "eft": bf(eftT),
            "onehot": bf(onehot),
            "nft": bf(nftT),
            "invc": invc,
            "poolw": bf(poolw),
            "g2aug": bf(g2aug),
            "gnaug": bf(gnaug),
            "globT": globT,
        }
        m.update(w_np)
        in_maps.append(m)

    return in_maps, NT, K0, pairs


# ---------------------------------------------------------------------------
# Device program (identical on all cores)


def _build(NT, K0):
    nc = bass.Bass()
    NPAD = NT * 128
    EPAD = NT * K0 * 128
    CW = K0 * 128  # edge columns per node-tile

    d_eft = nc.dram_tensor("eft", [128, EPAD], BF16, kind="ExternalInput")
    d_onehot = nc.dram_tensor("onehot", [128, EPAD], BF16, kind="ExternalInput")
    d_nft = nc.dram_tensor("nft", [128, NPAD], BF16, kind="ExternalInput")
    d_invc = nc.dram_tensor("invc", [NPAD, 1], F32, kind="ExternalInput")
    d_poolw = nc.dram_tensor("poolw", [NPAD, 128], BF16, kind="ExternalInput")
    d_g2aug = nc.dram_tensor("g2aug", [128, 128], BF16, kind="ExternalInput")
    d_gnaug = nc.dram_tensor("gnaug", [128, 128], BF16, kind="ExternalInput")
    d_globT = nc.dram_tensor("globT", [128, 2], F32, kind="ExternalInput")

    d_We1T = nc.dram_tensor("We1T", [128, 256], BF16, kind="ExternalInput")
    d_We2T = nc.dram_tensor("We2T", [128, 256], BF16, kind="ExternalInput")
    d_Wn1T = nc.dram_tensor("Wn1T", [128, 256], BF16, kind="ExternalInput")
    d_Win1T = nc.dram_tensor("Win1T", [128, 512], BF16, kind="ExternalInput")
    d_Wn2T = nc.dram_tensor("Wn2T", [128, 256], BF16, kind="ExternalInput")
    d_Win2T = nc.dram_tensor("Win2T", [128, 128], BF16, kind="ExternalInput")
    d_WgnT = nc.dram_tensor("WgnT", [128, 128], F32, kind="ExternalInput")
    d_WgeT = nc.dram_tensor("WgeT", [128, 128], F32, kind="ExternalInput")
    d_WggT = nc.dram_tensor("WggT", [128, 128], F32, kind="ExternalInput")
    d_bgr = nc.dram_tensor("bgr", [128, 128], F32, kind="ExternalInput")
    d_onesc = nc.dram_tensor("onesc", [128, 2], F32, kind="ExternalInput")
    d_identb = nc.dram_tensor("identb", [128, 128], BF16, kind="ExternalInput")
    d_ident4 = nc.dram_tensor("ident4", [4, 4], F32, kind="ExternalInput")

    d_out = nc.dram_tensor("out", [128, 2], F32, kind="ExternalOutput")

    Relu = mybir.ActivationFunctionType.Relu
    Copy = mybir.ActivationFunctionType.Copy
    NQ = K0 // 2   # e1 chunk-pairs per tile
    NR = K0 // 4   # e2 quads per tile
    NS = K0 // 2   # e1T 256-edge slabs per tile

    with tile.TileContext(nc) as tc:
        with tc.tile_pool(name="wp", bufs=1) as wp:
            def wtile(dram, shape, dt):
                t = wp.tile(shape, dt, tag=dram.name)
                nc.scalar.dma_start(t[:], dram[:])
                return t

            We1T = wtile(d_We1T, [128, 256], BF16)
            We2T = wtile(d_We2T, [128, 256], BF16)
            Wn1T = wtile(d_Wn1T, [128, 256], BF16)
            Win1T = wtile(d_Win1T, [128, 512], BF16)
            Wn2T = wtile(d_Wn2T, [128, 256], BF16)
            Win2T = wtile(d_Win2T, [128, 128], BF16)
            g2aug = wtile(d_g2aug, [128, 128], BF16)
            gnaug = wtile(d_gnaug, [128, 128], BF16)
            WgnT = wtile(d_WgnT, [128, 128], F32)
            WgeT = wtile(d_WgeT, [128, 128], F32)
            WggT = wtile(d_WggT, [128, 128], F32)
            bgr = wtile(d_bgr, [128, 128], F32)
            onesc = wtile(d_onesc, [128, 2], F32)
            identb = wtile(d_identb, [128, 128], BF16)
            ident4 = wtile(d_ident4, [4, 4], F32)
            globT = wtile(d_globT, [128, 2], F32)

            aggall = wp.tile([128, 384 * NT], BF16, tag="aggall")

            # ----------------- edge phase -----------------
            with tc.tile_pool(name="ep", bufs=3) as ep, \
                 tc.tile_pool(name="e1p", bufs=2) as e1p, \
                 tc.tile_pool(name="efp", bufs=NQ + 2) as efp, \
                 tc.tile_pool(name="psA", bufs=2, space=bass.MemorySpace.PSUM) as psA, \
                 tc.tile_pool(name="psB", bufs=2, space=bass.MemorySpace.PSUM) as psB, \
                 tc.tile_pool(name="psC", bufs=2, space=bass.MemorySpace.PSUM) as psC, \
                 tc.tile_pool(name="psAgg", bufs=2, space=bass.MemorySpace.PSUM) as psAgg:
                for t in range(NT):
                    eftt = ep.tile([128, CW], BF16, tag="eftt", bufs=2)
                    nc.sync.dma_start(eftt[:], d_eft[:, t * CW : (t + 1) * CW])
                    oht = ep.tile([128, CW], BF16, tag="oht", bufs=2)
                    nc.sync.dma_start(oht[:], d_onehot[:, t * CW : (t + 1) * CW])
                    invc_t = ep.tile([128, 1], F32, tag="invc")
                    nc.sync.dma_start(invc_t[:], d_invc[t * 128 : (t + 1) * 128, :])

                    # e1T: feat-major halves, 256-edge slabs
                    e1h0 = e1p.tile([128, CW], BF16, tag="e1h0")
                    e1h1 = e1p.tile([128, CW], BF16, tag="e1h1")
                    for s in range(NS):
                        sl = slice(s * 256, (s + 1) * 256)
                        pT = psA.tile([128, 512], F32, tag="pT")
                        nc.tensor.matmul(pT[:, 0:256], We1T[:, 0:128],
                                         eftt[:, sl], start=True, stop=True)
                        nc.tensor.matmul(pT[:, 256:512], We1T[:, 128:256],
                                         eftt[:, sl], start=True, stop=True)
                        nc.scalar.activation(e1h0[:, sl], pT[:, 0:256], Relu)
                        nc.vector.tensor_scalar_max(e1h1[:, sl], pT[:, 256:512], 0.0)

                    # e1 edge-major, 2-chunk pairs -> ef cols {0:256, 384:640}
                    efs = []
                    for q in range(NQ):
                        pE = psB.tile([128, 512], F32, tag="pE")
                        for j in (0, 1):
                            c = 2 * q + j
                            nc.tensor.matmul(pE[:, j * 256 : (j + 1) * 256],
                                             eftt[:, c * 128 : (c + 1) * 128],
                                             We1T[:], start=True, stop=True)
                        ef = efp.tile([128, 768], BF16, tag="ef")
                        dst = ef[:].rearrange("p (j x) -> p j x", j=2)[:, :, 0:256]
                        src = pE[:].rearrange("p (j x) -> p j x", j=2)
                        if q % 2 == 0:
                            nc.scalar.activation(dst, src, Relu)
                        else:
                            nc.vector.tensor_scalar_max(dst, src, 0.0)
                        efs.append(ef)

                    # e2 edge-major, 4-chunk quads -> ef cols {256:384, 640:768}
                    for r in range(NR):
                        pC = psC.tile([128, 512], F32, tag="pC")
                        for j in range(4):
                            c = 4 * r + j
                            csl = slice(c * 128, (c + 1) * 128)
                            osl = slice(j * 128, (j + 1) * 128)
                            nc.tensor.matmul(pC[:, osl], e1h0[:, csl],
                                             We2T[:, 0:128], start=True, stop=False)
                            nc.tensor.matmul(pC[:, osl], e1h1[:, csl],
                                             We2T[:, 128:256], start=False, stop=False)
                            nc.tensor.matmul(pC[:, osl], eftt[:, csl],
                                             g2aug[:], start=False, stop=True)
                        for h in (0, 1):
                            ef = efs[2 * r + h]
                            dst = ef[:].rearrange("p (j x) -> p j x", j=2)[:, :, 256:384]
                            src = pC[:, h * 256 : (h + 1) * 256].rearrange(
                                "p (j x) -> p j x", j=2)
                            nc.vector.tensor_scalar_max(dst, src, 0.0)

                    # aggregation: one N=384 matmul per chunk, single group
                    pagg = psAgg.tile([128, 384], F32, tag="pagg")
                    for c in range(K0):
                        csl = slice(c * 128, (c + 1) * 128)
                        nc.tensor.matmul(pagg[:], oht[:, csl],
                                         efs[c // 2][:, (c % 2) * 384 : (c % 2) * 384 + 384],
                                         start=(c == 0), stop=(c == K0 - 1))

                    nc.scalar.activation(
                        aggall[:, t * 384 : (t + 1) * 384], pagg[:], Copy,
                        scale=invc_t[:],
                    )

            # ----------------- node phase -----------------
            with tc.tile_pool(name="np_", bufs=NT) as np_, \
                 tc.tile_pool(name="agp", bufs=NT) as agp, \
                 tc.tile_pool(name="nsb", bufs=3) as nsb, \
                 tc.tile_pool(name="npsT", bufs=2, space=bass.MemorySpace.PSUM) as npsT, \
                 tc.tile_pool(name="npsB", bufs=2, space=bass.MemorySpace.PSUM) as npsB, \
                 tc.tile_pool(name="npsC", bufs=1, space=bass.MemorySpace.PSUM) as npsC, \
                 tc.tile_pool(name="npsP", bufs=1, space=bass.MemorySpace.PSUM) as npsP:
                ppN = npsP.tile([128, 128], F32, tag="ppN")
                ppE = npsP.tile([128, 128], F32, tag="ppE")

                nftts, pws = [], []
                for t in range(NT):
                    nftt = np_.tile([128, 128], BF16, tag="nftt")
                    nc.sync.dma_start(nftt[:], d_nft[:, t * 128 : (t + 1) * 128])
                    pw = np_.tile([128, 128], BF16, tag="pw")
                    nc.sync.dma_start(pw[:], d_poolw[t * 128 : (t + 1) * 128, :])
                    nftts.append(nftt)
                    pws.append(pw)

                # pass 1: transpose agg tiles to feat-major
                aggTs = []
                for t in range(NT):
                    aggsl = aggall[:, t * 384 : (t + 1) * 384]
                    pT = npsT.tile([128, 384], BF16, tag="pT")
                    nc.tensor.transpose(pT[:, 0:128], aggsl[:, 0:128], identb[:])
                    nc.tensor.transpose(pT[:, 128:256], aggsl[:, 128:256], identb[:])
                    nc.tensor.transpose(pT[:, 256:384], aggsl[:, 256:384], identb[:])
                    aggT = agp.tile([128, 384], BF16, tag="aggT")
                    nc.vector.tensor_copy(aggT[:], pT[:])
                    aggTs.append(aggT)

                # pass 2: node MLPs + pooling
                for t in range(NT):
                    aggsl = aggall[:, t * 384 : (t + 1) * 384]
                    aggT = aggTs[t]
                    nftt = nftts[t]
                    pw = pws[t]

                    pn1 = npsB.tile([128, 256], F32, tag="pn1")
                    for s in (0, 1):
                        ssl = slice(s * 128, (s + 1) * 128)
                        nc.tensor.matmul(pn1[:, ssl], Wn1T[:, ssl], nftt[:],
                                         start=True, stop=False)
                        nc.tensor.matmul(pn1[:, ssl], Win1T[:, s * 128 : s * 128 + 128],
                                         aggT[:, 0:128], start=False, stop=False)
                        nc.tensor.matmul(pn1[:, ssl], Win1T[:, 256 + s * 128 : 256 + s * 128 + 128],
                                         aggT[:, 128:256], start=False, stop=True)
                    n1T = nsb.tile([128, 256], BF16, tag="n1T")
                    nc.scalar.activation(n1T[:], pn1[:], Relu)

                    pn2 = npsC.tile([128, 128], F32, tag="pn2")
                    nc.tensor.matmul(pn2[:], n1T[:, 0:128], Wn2T[:, 0:128], start=True, stop=False)
                    nc.tensor.matmul(pn2[:], n1T[:, 128:256], Wn2T[:, 128:256], start=False, stop=False)
                    nc.tensor.matmul(pn2[:], aggT[:, 256:384], Win2T[:], start=False, stop=False)
                    nc.tensor.matmul(pn2[:], nftt[:], gnaug[:], start=False, stop=True)
                    n2 = nsb.tile([128, 128], BF16, tag="n2")
                    nc.vector.tensor_scalar_max(n2[:], pn2[:], 0.0)

                    nc.tensor.matmul(ppN[:], pw[:], n2[:],
                                     start=(t == 0), stop=(t == NT - 1))
                    nc.tensor.matmul(ppE[:], pw[:], aggsl[:, 256:384],
                                     start=(t == 0), stop=(t == NT - 1))

                # ----------------- final projection -----------------
                pp4 = nsb.tile([4, 256], F32, tag="pp4")
                nc.scalar.activation(pp4[:, 0:128], ppN[0:4, :], Copy)
                nc.scalar.activation(pp4[:, 128:256], ppE[0:4, :], Copy)

                ptail = npsP.tile([128, 16], F32, tag="ptail")
                nc.tensor.transpose(ptail[:, 0:4], pp4[:, 0:128], ident4[:])
                nc.tensor.transpose(ptail[:, 4:8], pp4[:, 128:256], ident4[:])
                nt8 = nsb.tile([128, 8], F32, tag="nt8")
                nc.scalar.activation(nt8[:], ptail[:, 0:8], Copy)

                nc.tensor.matmul(ptail[:, 8:10], WgnT[:], nt8[:, 0:2], start=True, stop=False)
                nc.tensor.matmul(ptail[:, 8:10], WgeT[:], nt8[:, 6:8], start=False, stop=False)
                nc.tensor.matmul(ptail[:, 8:10], WggT[:], globT[:], start=False, stop=False)
                nc.tensor.matmul(ptail[:, 8:10], bgr[:], onesc[:], start=False, stop=True)
                outsb = nsb.tile([128, 2], F32, tag="outsb")
                nc.scalar.activation(outsb[:], ptail[:, 8:10], Copy)
                nc.sync.dma_start(d_out[:], outsb[:])

    return nc


_CACHE = {}


def _get_nc(NT, K0):
    key = (NT, K0)
    if key not in _CACHE:
        _CACHE[key] = _build(NT, K0)
    return _CACHE[key]


def _run(inputs, trace=False):
    in_maps, NT, K0, pairs = _prepare(inputs)
    nc = _get_nc(NT, K0)
    res = run_bass_kernel_spmd(nc, in_maps, list(range(N_CORES)), trace=trace)
    out = np.zeros((N_GRAPHS, 128), np.float32)
    for c in range(N_CORES):
        r = np.asarray(res.results[c]["out"], np.float32)
        pa, pb = pairs[c]
        out[pa] = r[:, 0]
        out[pb] = r[:, 1]
    return out, res


def kernel(**inputs):
    out, _ = _run(inputs, trace=False)
    return out


def kernel_traced(**inputs):
    return _run(inputs, trace=True)


# revision 9
# speedup vs baseline: 2.2156x; 1.0063x over previous
"""Trainium2 Bass kernel for a 2-layer GraphNetwork (gnn_message_passing).

Strategy (v2):
  - 16 graphs partitioned across 8 cores, 2 graphs per core, paired
    big+small by edge count to balance load. All segment reductions are
    core-local; [16,128] output rows are gathered on the host.
  - Per core, nodes are bin-packed (LPT) into NT tiles of 128 slots; each
    tile's incoming edges are padded to K0 chunks of 128. Segment sums run
    on the tensor engine as one-hot matmuls with HOST-built one-hot tiles.
  - Every hot-loop matmul uses a full K=128 stationary: edge/node feature
    tiles are zero-padded to 128 partitions, with ones/graph-indicator
    rows folded in so biases and global-feature terms are matmul
    accumulations against padded weight tiles. (Partial-K matmuls throttle
    the PE clock to 1.2 GHz; full-K keeps it at 2.4 GHz.)
  - bf16 inputs/intermediates, fp32 PSUM accumulation, fp32 final stage.
"""

import numpy as np
import ml_dtypes

import concourse.bass as bass
import concourse.tile as tile_mod
from concourse import tile
from concourse.bass_utils import run_bass_kernel_spmd
from concourse.vector_clock import ScopedClock

mybir = bass.mybir

N_NODES, N_EDGES, N_GRAPHS = 20000, 320000, 16
F_NODE, F_EDGE, F_GLOB = 64, 32, 16
N_CORES = 8
GPC = N_GRAPHS // N_CORES  # graphs per core = 2

BF16 = mybir.dt.bfloat16
FP8 = mybir.dt.float8e4
F32 = mybir.dt.float32
npbf16 = ml_dtypes.bfloat16
npfp8 = ml_dtypes.float8_e4m3

# ---------------------------------------------------------------------------
# Workaround: CoreV3 codegen rejects instructions carrying more than one
# semaphore wait. Split the waits across extra no-ops.
_MAX_WAITS = 1
_ENGINE_WAIT_LIMIT = 1
_SPLIT_ENGINES = None  # set lazily


def _split_excess_waits(nc):
    global _SPLIT_ENGINES
    if _SPLIT_ENGINES is None:
        ET = mybir.EngineType
        _SPLIT_ENGINES = {ET.PE, ET.Activation, ET.DVE, ET.SP, ET.Pool}
    ctr = [0]
    for bass_bb in nc.bb_map.values():
        bb = bass_bb.bb
        il = bb.instructions
        out = []
        changed = False
        for inst in il:
            si = inst.sync_info
            waits = list(si.on_wait) if (si and si.on_wait) else []
            if len(waits) > _ENGINE_WAIT_LIMIT and inst.engine in _SPLIT_ENGINES:
                head, keep = waits[:-_ENGINE_WAIT_LIMIT], waits[-_ENGINE_WAIT_LIMIT:]
                for i in range(0, len(head), _ENGINE_WAIT_LIMIT):
                    nop = mybir.InstNoOp(name=f"waitsplit-{ctr[0]}", ins=[], outs=[])
                    ctr[0] += 1
                    nop.engine = inst.engine
                    nop.sync_info = mybir.SyncInfo(
                        on_wait=head[i : i + _ENGINE_WAIT_LIMIT], on_update=[]
                    )
                    nc.register_instruction(nop, overwrite=True)
                    out.append(nop)
                inst.sync_info = mybir.SyncInfo(
                    on_wait=keep, on_update=list(si.on_update or [])
                )
                changed = True
            out.append(inst)
        if changed:
            bb.instructions = out


def _split_drain_and_barrier(self, tick_clock, wait_clock):
    nc = self.nc
    _split_excess_waits(nc)
    drain_inst = nc.sync.drain()
    wait_clock.add_sem_waits(
        drain_inst.ins, ScopedClock({None: tick_clock.global_clock})
    )
    mi = drain_inst.ins
    waits = list(mi.sync_info.on_wait) if (mi.sync_info and mi.sync_info.on_wait) else []
    if len(waits) > _MAX_WAITS:
        upd = list(mi.sync_info.on_update) if mi.sync_info.on_update else []
        mi.sync_info = mybir.SyncInfo(on_wait=waits[:_MAX_WAITS], on_update=upd)
        for i in range(_MAX_WAITS, len(waits), _MAX_WAITS):
            nop = nc.sync.nop(nofuse=True)
            nop.ins.sync_info = mybir.SyncInfo(
                on_wait=waits[i : i + _MAX_WAITS], on_update=[]
            )
    nc.all_engine_barrier()
    assert self.sems is not None
    popped = nc._tile_sem_poison_stack.pop()
    assert popped is self._sem_poison
    nc.clear_and_free_semaphores(list(self.sems.allocated().values()))
    nc.all_engine_barrier()


tile_mod.TileContext._drain_and_barrier = _split_drain_and_barrier


# ---------------------------------------------------------------------------
# Host-side graph partitioning / layout


def _pack_core(node_ids, degs, nt, cap_e):
    """LPT: place nodes (descending degree) onto the least-edge-loaded tile
    that still has node capacity. Returns per-tile node-id arrays, or None
    if some tile exceeds cap_e edges."""
    order = np.argsort(-degs, kind="stable")
    tiles_n = [[] for _ in range(nt)]
    tile_ncnt = np.zeros(nt, np.int64)
    tile_ecnt = np.zeros(nt, np.int64)
    for j in order:
        cand = np.where(tile_ncnt < 128)[0]
        if len(cand) == 0:
            return None
        t = cand[np.argmin(tile_ecnt[cand])]
        tiles_n[t].append(node_ids[j])
        tile_ncnt[t] += 1
        tile_ecnt[t] += degs[j]
    if (tile_ecnt > cap_e).any():
        return None
    return [np.array(t, dtype=np.int64) for t in tiles_n]


def _prepare(inputs):
    nf = np.asarray(inputs["node_feats"], np.float32)
    ef = np.asarray(inputs["edge_feats"], np.float32)
    glob = np.asarray(inputs["globals_"], np.float32)
    recv = np.asarray(inputs["receivers"]).astype(np.int64)
    ngraph = np.asarray(inputs["node_graph"]).astype(np.int64)

    cnt = np.bincount(recv, minlength=N_NODES).astype(np.int64)
    egraph = ngraph[recv]
    ncnt_g = np.bincount(ngraph, minlength=N_GRAPHS)
    ecnt_g = np.bincount(egraph, minlength=N_GRAPHS)

    # pair graphs big+small by edge count to balance cores
    order = np.argsort(-ecnt_g, kind="stable")
    pairs = [(int(order[i]), int(order[N_GRAPHS - 1 - i])) for i in range(N_CORES)]

    core_nodes = [
        np.where((ngraph == pa) | (ngraph == pb))[0] for pa, pb in pairs
    ]
    NT = int(max((len(cn) + 127) // 128 for cn in core_nodes))

    core_of_graph = np.zeros(N_GRAPHS, np.int64)
    for c, (pa, pb) in enumerate(pairs):
        core_of_graph[pa] = c
        core_of_graph[pb] = c
    edge_core = core_of_graph[egraph]
    ecnt_core = np.bincount(edge_core, minlength=N_CORES)

    packs = None
    K0 = max(1, int(max(ecnt_core) + NT * 128 - 1) // (NT * 128))
    K0 = ((K0 + 3) // 4) * 4  # quad-chunk device loops need K0 % 4 == 0
    for k0 in range(K0, K0 + 12, 4):
        trial = []
        ok = True
        for c in range(N_CORES):
            p = _pack_core(core_nodes[c], cnt[core_nodes[c]], NT, k0 * 128)
            if p is None:
                ok = False
                break
            trial.append(p)
        if ok:
            packs, K0 = trial, k0
            break
    assert packs is not None, "bin packing failed"

    NPAD = NT * 128
    EPAD = NT * K0 * 128

    # replicated weights
    w_np = {}

    def bf(x):
        return np.ascontiguousarray(x).astype(npbf16)

    We1T = np.zeros((128, 256), np.float32)
    We1T[:32] = np.asarray(inputs["We1"], np.float32).T
    We1T[32] = np.asarray(inputs["be1"], np.float32)
    w_np["We1T"] = bf(We1T)

    We2T = np.asarray(inputs["We2"], np.float32).T  # [256, 128]
    w_np["We2T"] = bf(np.concatenate([We2T[:128], We2T[128:]], axis=1))  # [128,256]

    Wn1T = np.zeros((128, 256), np.float32)
    Wn1T[:64] = np.asarray(inputs["Wn1"], np.float32).T
    Wn1T[64] = np.asarray(inputs["bn1"], np.float32)
    w_np["Wn1T"] = bf(Wn1T)

    Win1T = np.asarray(inputs["Win1"], np.float32).T  # [256 fi, 256 fo]
    w_np["Win1T"] = bf(
        np.concatenate(
            [Win1T[:128, :128], Win1T[:128, 128:], Win1T[128:, :128], Win1T[128:, 128:]],
            axis=1,
        )
    )  # [128, 512]

    Wn2T = np.asarray(inputs["Wn2"], np.float32).T  # [256, 128]
    w_np["Wn2T"] = bf(np.concatenate([Wn2T[:128], Wn2T[128:]], axis=1))  # [128,256]
    w_np["Win2T"] = bf(np.asarray(inputs["Win2"], np.float32).T)  # [128, 128]

    WggT_pad = np.zeros((128, 128), np.float32)
    WggT_pad[:16] = np.asarray(inputs["Wgg"], np.float32).T
    w_np["WgnT"] = np.asarray(inputs["Wgn"], np.float32).T.copy()
    w_np["WgeT"] = np.asarray(inputs["Wge"], np.float32).T.copy()
    w_np["WggT"] = WggT_pad
    bgr_pad = np.zeros((128, 128), np.float32)
    bgr_pad[0] = np.asarray(inputs["bg"], np.float32)
    w_np["bgr"] = bgr_pad
    onesc = np.zeros((128, 2), np.float32)
    onesc[0] = 1.0
    w_np["onesc"] = onesc
    w_np["identb"] = np.eye(128, dtype=npbf16)
    w_np["ident4"] = np.eye(4, dtype=np.float32)

    Wg2 = np.asarray(inputs["Wg2"], np.float32)  # [128, 16]
    Wng2 = np.asarray(inputs["Wng2"], np.float32)
    be2 = np.asarray(inputs["be2"], np.float32)
    bn2 = np.asarray(inputs["bn2"], np.float32)

    # per-core inputs
    in_maps = []
    slot_of_node = np.full(N_NODES, -1, np.int64)
    tile_of_node = np.full(N_NODES, -1, np.int64)
    for c in range(N_CORES):
        pa, pb = pairs[c]
        for t in range(NT):
            ids = packs[c][t]
            slot_of_node[ids] = t * 128 + np.arange(len(ids))
            tile_of_node[ids] = t

        # ---- edges
        eidx = np.where(edge_core == c)[0]
        et = tile_of_node[recv[eidx]]
        order_e = np.argsort(et, kind="stable")
        eidx = eidx[order_e]
        et = et[order_e]
        counts = np.bincount(et, minlength=NT)
        starts = np.concatenate([[0], np.cumsum(counts)[:-1]])
        off_in = np.arange(len(eidx)) - np.repeat(starts, counts)
        dst = et * (K0 * 128) + off_in
        assert (counts <= K0 * 128).all()

        eftT = np.zeros((128, EPAD), np.float32)
        eftT[:32, dst] = ef[eidx].T
        eftT[32, dst] = 1.0
        eg = egraph[eidx]
        eftT[33, dst] = (eg == pa)
        eftT[34, dst] = (eg == pb)

        # one-hot: per 128-edge chunk a [lane, slot] block, K(partitions)=lanes
        onehot = np.zeros((128, EPAD), np.float32)
        onehot[dst % 128, (dst // 128) * 128 + slot_of_node[recv[eidx]] % 128] = 1.0

        # ---- nodes
        slot_node = np.full(NPAD, -1, np.int64)
        for t in range(NT):
            ids = packs[c][t]
            slot_node[t * 128 : t * 128 + len(ids)] = ids
        valid = slot_node >= 0
        sn = np.where(valid, slot_node, 0)

        nftT = np.zeros((128, NPAD), np.float32)
        nftT[:64, valid] = nf[sn[valid]].T
        nftT[64, valid] = 1.0
        ng = ngraph[sn]
        nftT[65] = valid * (ng == pa)
        nftT[66] = valid * (ng == pb)

        invc = np.ones((NPAD, 1), np.float32)
        invc[valid, 0] = 1.0 / np.maximum(cnt[sn[valid]], 1)

        poolw = np.zeros((NPAD, 128), np.float32)
        for g, gid in enumerate((pa, pb)):
            m = valid & (ng == gid)
            poolw[m, g] = 1.0 / max(ncnt_g[gid], 1)
            poolw[m, 2 + g] = cnt[sn[m]] / max(ecnt_g[gid], 1)

        g2aug = np.zeros((128, 128), np.float32)
        g2aug[32] = be2
        g2aug[33] = Wg2 @ glob[pa]
        g2aug[34] = Wg2 @ glob[pb]

        gnaug = np.zeros((128, 128), np.float32)
        gnaug[64] = bn2
        gnaug[65] = Wng2 @ glob[pa]
        gnaug[66] = Wng2 @ glob[pb]

        globT = np.zeros((128, 2), np.float32)
        globT[:16, 0] = glob[pa]
        globT[:16, 1] = glob[pb]

        m = {
            "eft": bf(eftT),
            "onehot": np.ascontiguousarray(onehot).astype(npfp8),
            "nft": bf(nftT),
            "invc": invc,
            "poolw": bf(poolw),
            "g2aug": bf(g2aug),
            "gnaug": bf(gnaug),
            "globT": globT,
        }
        m.update(w_np)
        in_maps.append(m)

    return in_maps, NT, K0, pairs


# ---------------------------------------------------------------------------
# Device program (identical on all cores)


def _build(NT, K0):
    nc = bass.Bass()
    NPAD = NT * 128
    EPAD = NT * K0 * 128
    CW = K0 * 128  # edge columns per node-tile

    d_eft = nc.dram_tensor("eft", [128, EPAD], BF16, kind="ExternalInput")
    d_onehot = nc.dram_tensor("onehot", [128, EPAD], FP8, kind="ExternalInput")
    d_nft = nc.dram_tensor("nft", [128, NPAD], BF16, kind="ExternalInput")
    d_invc = nc.dram_tensor("invc", [NPAD, 1], F32, kind="ExternalInput")
    d_poolw = nc.dram_tensor("poolw", [NPAD, 128], BF16, kind="ExternalInput")
    d_g2aug = nc.dram_tensor("g2aug", [128, 128], BF16, kind="ExternalInput")
    d_gnaug = nc.dram_tensor("gnaug", [128, 128], BF16, kind="ExternalInput")
    d_globT = nc.dram_tensor("globT", [128, 2], F32, kind="ExternalInput")

    d_We1T = nc.dram_tensor("We1T", [128, 256], BF16, kind="ExternalInput")
    d_We2T = nc.dram_tensor("We2T", [128, 256], BF16, kind="ExternalInput")
    d_Wn1T = nc.dram_tensor("Wn1T", [128, 256], BF16, kind="ExternalInput")
    d_Win1T = nc.dram_tensor("Win1T", [128, 512], BF16, kind="ExternalInput")
    d_Wn2T = nc.dram_tensor("Wn2T", [128, 256], BF16, kind="ExternalInput")
    d_Win2T = nc.dram_tensor("Win2T", [128, 128], BF16, kind="ExternalInput")
    d_WgnT = nc.dram_tensor("WgnT", [128, 128], F32, kind="ExternalInput")
    d_WgeT = nc.dram_tensor("WgeT", [128, 128], F32, kind="ExternalInput")
    d_WggT = nc.dram_tensor("WggT", [128, 128], F32, kind="ExternalInput")
    d_bgr = nc.dram_tensor("bgr", [128, 128], F32, kind="ExternalInput")
    d_onesc = nc.dram_tensor("onesc", [128, 2], F32, kind="ExternalInput")
    d_identb = nc.dram_tensor("identb", [128, 128], BF16, kind="ExternalInput")
    d_ident4 = nc.dram_tensor("ident4", [4, 4], F32, kind="ExternalInput")

    d_out = nc.dram_tensor("out", [128, 2], F32, kind="ExternalOutput")

    Relu = mybir.ActivationFunctionType.Relu
    Copy = mybir.ActivationFunctionType.Copy
    NQ = K0 // 2   # e1 chunk-pairs per tile
    NR = K0 // 4   # e2 quads per tile
    NS = K0 // 2   # e1T 256-edge slabs per tile

    with tile.TileContext(nc) as tc:
        with tc.tile_pool(name="wp", bufs=1) as wp:
            def wtile(dram, shape, dt):
                t = wp.tile(shape, dt, tag=dram.name)
                nc.scalar.dma_start(t[:], dram[:])
                return t

            We1T = wtile(d_We1T, [128, 256], BF16)
            We2T = wtile(d_We2T, [128, 256], BF16)
            Wn1T = wtile(d_Wn1T, [128, 256], BF16)
            Win1T = wtile(d_Win1T, [128, 512], BF16)
            Wn2T = wtile(d_Wn2T, [128, 256], BF16)
            Win2T = wtile(d_Win2T, [128, 128], BF16)
            g2aug = wtile(d_g2aug, [128, 128], BF16)
            gnaug = wtile(d_gnaug, [128, 128], BF16)
            WgnT = wtile(d_WgnT, [128, 128], F32)
            WgeT = wtile(d_WgeT, [128, 128], F32)
            WggT = wtile(d_WggT, [128, 128], F32)
            bgr = wtile(d_bgr, [128, 128], F32)
            onesc = wtile(d_onesc, [128, 2], F32)
            identb = wtile(d_identb, [128, 128], BF16)
            ident4 = wtile(d_ident4, [4, 4], F32)
            globT = wtile(d_globT, [128, 2], F32)

            aggall = wp.tile([128, 384 * NT], BF16, tag="aggall")

            # ----------------- edge phase -----------------
            with tc.tile_pool(name="ep", bufs=3) as ep, \
                 tc.tile_pool(name="e1p", bufs=2) as e1p, \
                 tc.tile_pool(name="efp", bufs=NQ + 2) as efp, \
                 tc.tile_pool(name="psA", bufs=2, space=bass.MemorySpace.PSUM) as psA, \
                 tc.tile_pool(name="psB", bufs=2, space=bass.MemorySpace.PSUM) as psB, \
                 tc.tile_pool(name="psC", bufs=2, space=bass.MemorySpace.PSUM) as psC, \
                 tc.tile_pool(name="psAgg", bufs=2, space=bass.MemorySpace.PSUM) as psAgg:
                for t in range(NT):
                    eftt = ep.tile([128, CW], BF16, tag="eftt", bufs=2)
                    nc.sync.dma_start(eftt[:], d_eft[:, t * CW : (t + 1) * CW])
                    oht = ep.tile([128, CW], FP8, tag="oht", bufs=2)
                    nc.sync.dma_start(oht[:], d_onehot[:, t * CW : (t + 1) * CW])
                    invc_t = ep.tile([128, 1], F32, tag="invc")
                    nc.sync.dma_start(invc_t[:], d_invc[t * 128 : (t + 1) * 128, :])

                    # e1T: feat-major halves, 256-edge slabs
                    e1h0 = e1p.tile([128, CW], BF16, tag="e1h0")
                    e1h1 = e1p.tile([128, CW], BF16, tag="e1h1")
                    for s in range(NS):
                        sl = slice(s * 256, (s + 1) * 256)
                        pT = psA.tile([128, 512], F32, tag="pT")
                        nc.tensor.matmul(pT[:, 0:256], We1T[:, 0:128],
                                         eftt[:, sl], start=True, stop=True)
                        nc.tensor.matmul(pT[:, 256:512], We1T[:, 128:256],
                                         eftt[:, sl], start=True, stop=True)
                        nc.scalar.activation(e1h0[:, sl], pT[:, 0:256], Relu)
                        nc.vector.tensor_scalar_max(e1h1[:, sl], pT[:, 256:512], 0.0)

                    # e1 edge-major, 2-chunk pairs -> ef cols {0:256, 384:640}
                    efs = []
                    for q in range(NQ):
                        pE = psB.tile([128, 512], F32, tag="pE")
                        for j in (0, 1):
                            c = 2 * q + j
                            nc.tensor.matmul(pE[:, j * 256 : (j + 1) * 256],
                                             eftt[:, c * 128 : (c + 1) * 128],
                                             We1T[:], start=True, stop=True)
                        ef = efp.tile([128, 768], FP8, tag="ef")
                        dst = ef[:].rearrange("p (j x) -> p j x", j=2)[:, :, 0:256]
                        src = pE[:].rearrange("p (j x) -> p j x", j=2)
                        if q % 2 == 0:
                            nc.scalar.activation(dst, src, Relu)
                        else:
                            nc.vector.tensor_scalar_max(dst, src, 0.0)
                        efs.append(ef)

                    # e2 edge-major, 4-chunk quads -> ef cols {256:384, 640:768}
                    for r in range(NR):
                        pC = psC.tile([128, 512], F32, tag="pC")
                        for j in range(4):
                            c = 4 * r + j
                            csl = slice(c * 128, (c + 1) * 128)
                            osl = slice(j * 128, (j + 1) * 128)
                            nc.tensor.matmul(pC[:, osl], e1h0[:, csl],
                                             We2T[:, 0:128], start=True, stop=False)
                            nc.tensor.matmul(pC[:, osl], e1h1[:, csl],
                                             We2T[:, 128:256], start=False, stop=False)
                            nc.tensor.matmul(pC[:, osl], eftt[:, csl],
                                             g2aug[:], start=False, stop=True)
                        for h in (0, 1):
                            ef = efs[2 * r + h]
                            dst = ef[:].rearrange("p (j x) -> p j x", j=2)[:, :, 256:384]
                            src = pC[:, h * 256 : (h + 1) * 256].rearrange(
                                "p (j x) -> p j x", j=2)
                            nc.vector.tensor_scalar_max(dst, src, 0.0)

                    # aggregation: fp8 DoubleRow, 256 edges (2 chunks) per matmul
                    pagg = psAgg.tile([128, 384], F32, tag="pagg")
                    for q in range(NQ):
                        lhs3 = oht[:, q * 256 : (q + 1) * 256].rearrange(
                            "k (p m) -> k p m", p=2)
                        rhs3 = efs[q][:].rearrange("k (p n) -> k p n", p=2)
                        nc.tensor.matmul(pagg[:], lhs3, rhs3,
                                         start=(q == 0), stop=(q == NQ - 1),
                                         perf_mode=mybir.MatmulPerfMode.DoubleRow)

                    nc.scalar.activation(
                        aggall[:, t * 384 : (t + 1) * 384], pagg[:], Copy,
                        scale=invc_t[:],
                    )

            # ----------------- node phase -----------------
            with tc.tile_pool(name="np_", bufs=NT) as np_, \
                 tc.tile_pool(name="agp", bufs=NT) as agp, \
                 tc.tile_pool(name="nsb", bufs=3) as nsb, \
                 tc.tile_pool(name="npsT", bufs=2, space=bass.MemorySpace.PSUM) as npsT, \
                 tc.tile_pool(name="npsB", bufs=2, space=bass.MemorySpace.PSUM) as npsB, \
                 tc.tile_pool(name="npsC", bufs=1, space=bass.MemorySpace.PSUM) as npsC, \
                 tc.tile_pool(name="npsP", bufs=1, space=bass.MemorySpace.PSUM) as npsP:
                ppN = npsP.tile([128, 128], F32, tag="ppN")
                ppE = npsP.tile([128, 128], F32, tag="ppE")

                nftts, pws = [], []
                for t in range(NT):
                    nftt = np_.tile([128, 128], BF16, tag="nftt")
                    nc.sync.dma_start(nftt[:], d_nft[:, t * 128 : (t + 1) * 128])
                    pw = np_.tile([128, 128], BF16, tag="pw")
                    nc.sync.dma_start(pw[:], d_poolw[t * 128 : (t + 1) * 128, :])
                    nftts.append(nftt)
                    pws.append(pw)

                # pass 1: transpose agg tiles to feat-major
                aggTs = []
                for t in range(NT):
                    aggsl = aggall[:, t * 384 : (t + 1) * 384]
                    pT = npsT.tile([128, 384], BF16, tag="pT")
                    nc.tensor.transpose(pT[:, 0:128], aggsl[:, 0:128], identb[:])
                    nc.tensor.transpose(pT[:, 128:256], aggsl[:, 128:256], identb[:])
                    nc.tensor.transpose(pT[:, 256:384], aggsl[:, 256:384], identb[:])
                    aggT = agp.tile([128, 384], BF16, tag="aggT")
                    nc.vector.tensor_copy(aggT[:], pT[:])
                    aggTs.append(aggT)

                # pass 2: node MLPs + pooling
                for t in range(NT):
                    aggsl = aggall[:, t * 384 : (t + 1) * 384]
                    aggT = aggTs[t]
                    nftt = nftts[t]
                    pw = pws[t]

                    pn1 = npsB.tile([128, 256], F32, tag="pn1")
                    for s in (0, 1):
                        ssl = slice(s * 128, (s + 1) * 128)
                        nc.tensor.matmul(pn1[:, ssl], Wn1T[:, ssl], nftt[:],
                                         start=True, stop=False)
                        nc.tensor.matmul(pn1[:, ssl], Win1T[:, s * 128 : s * 128 + 128],
                                         aggT[:, 0:128], start=False, stop=False)
                        nc.tensor.matmul(pn1[:, ssl], Win1T[:, 256 + s * 128 : 256 + s * 128 + 128],
                                         aggT[:, 128:256], start=False, stop=True)
                    n1T = nsb.tile([128, 256], BF16, tag="n1T")
                    nc.scalar.activation(n1T[:], pn1[:], Relu)

                    pn2 = npsC.tile([128, 128], F32, tag="pn2")
                    nc.tensor.matmul(pn2[:], n1T[:, 0:128], Wn2T[:, 0:128], start=True, stop=False)
                    nc.tensor.matmul(pn2[:], n1T[:, 128:256], Wn2T[:, 128:256], start=False, stop=False)
                    nc.tensor.matmul(pn2[:], aggT[:, 256:384], Win2T[:], start=False, stop=False)
                    nc.tensor.matmul(pn2[:], nftt[:], gnaug[:], start=False, stop=True)
                    n2 = nsb.tile([128, 128], BF16, tag="n2")
                    nc.vector.tensor_scalar_max(n2[:], pn2[:], 0.0)

                    nc.tensor.matmul(ppN[:], pw[:], n2[:],
                                     start=(t == 0), stop=(t == NT - 1))
                    nc.tensor.matmul(ppE[:], pw[:], aggsl[:, 256:384],
                                     start=(t == 0), stop=(t == NT - 1))

                # ----------------- final projection -----------------
                pp4 = nsb.tile([4, 256], F32, tag="pp4")
                nc.scalar.activation(pp4[:, 0:128], ppN[0:4, :], Copy)
                nc.scalar.activation(pp4[:, 128:256], ppE[0:4, :], Copy)

                ptail = npsP.tile([128, 16], F32, tag="ptail")
                nc.tensor.transpose(ptail[:, 0:4], pp4[:, 0:128], ident4[:])
                nc.tensor.transpose(ptail[:, 4:8], pp4[:, 128:256], ident4[:])
                nt8 = nsb.tile([128, 8], F32, tag="nt8")
                nc.scalar.activation(nt8[:], ptail[:, 0:8], Copy)

                nc.tensor.matmul(ptail[:, 8:10], WgnT[:], nt8[:, 0:2], start=True, stop=False)
                nc.tensor.matmul(ptail[:, 8:10], WgeT[:], nt8[:, 6:8], start=False, stop=False)
                nc.tensor.matmul(ptail[:, 8:10], WggT[:], globT[:], start=False, stop=False)
                nc.tensor.matmul(ptail[:, 8:10], bgr[:], onesc[:], start=False, stop=True)
                outsb = nsb.tile([128, 2], F32, tag="outsb")
                nc.scalar.activation(outsb[:], ptail[:, 8:10], Copy)
                nc.sync.dma_start(d_out[:], outsb[:])

    return nc


_CACHE = {}


def _get_nc(NT, K0):
    key = (NT, K0)
    if key not in _CACHE:
        _CACHE[key] = _build(NT, K0)
    return _CACHE[key]


def _run(inputs, trace=False):
    in_maps, NT, K0, pairs = _prepare(inputs)
    nc = _get_nc(NT, K0)
    res = run_bass_kernel_spmd(nc, in_maps, list(range(N_CORES)), trace=trace)
    out = np.zeros((N_GRAPHS, 128), np.float32)
    for c in range(N_CORES):
        r = np.asarray(res.results[c]["out"], np.float32)
        pa, pb = pairs[c]
        out[pa] = r[:, 0]
        out[pb] = r[:, 1]
    return out, res


def kernel(**inputs):
    out, _ = _run(inputs, trace=False)
    return out


def kernel_traced(**inputs):
    return _run(inputs, trace=True)


# revision 10
# speedup vs baseline: 2.5076x; 1.1318x over previous
"""Trainium2 Bass kernel for a 2-layer GraphNetwork (gnn_message_passing).

Strategy (v2):
  - 16 graphs partitioned across 8 cores, 2 graphs per core, paired
    big+small by edge count to balance load. All segment reductions are
    core-local; [16,128] output rows are gathered on the host.
  - Per core, nodes are bin-packed (LPT) into NT tiles of 128 slots; each
    tile's incoming edges are padded to K0 chunks of 128. Segment sums run
    on the tensor engine as one-hot matmuls with HOST-built one-hot tiles.
  - Every hot-loop matmul uses a full K=128 stationary: edge/node feature
    tiles are zero-padded to 128 partitions, with ones/graph-indicator
    rows folded in so biases and global-feature terms are matmul
    accumulations against padded weight tiles. (Partial-K matmuls throttle
    the PE clock to 1.2 GHz; full-K keeps it at 2.4 GHz.)
  - bf16 inputs/intermediates, fp32 PSUM accumulation, fp32 final stage.
"""

import numpy as np
import ml_dtypes

import concourse.bass as bass
import concourse.tile as tile_mod
from concourse import tile
from concourse.bass_utils import run_bass_kernel_spmd
from concourse.vector_clock import ScopedClock

mybir = bass.mybir

N_NODES, N_EDGES, N_GRAPHS = 20000, 320000, 16
F_NODE, F_EDGE, F_GLOB = 64, 32, 16
N_CORES = 8
GPC = N_GRAPHS // N_CORES  # graphs per core = 2

BF16 = mybir.dt.bfloat16
FP8 = mybir.dt.float8e4
F32 = mybir.dt.float32
npbf16 = ml_dtypes.bfloat16
npfp8 = ml_dtypes.float8_e4m3

# ---------------------------------------------------------------------------
# Workaround: CoreV3 codegen rejects instructions carrying more than one
# semaphore wait. Split the waits across extra no-ops.
_MAX_WAITS = 1
_ENGINE_WAIT_LIMIT = 1
_SPLIT_ENGINES = None  # set lazily


def _split_excess_waits(nc):
    global _SPLIT_ENGINES
    if _SPLIT_ENGINES is None:
        ET = mybir.EngineType
        _SPLIT_ENGINES = {ET.PE, ET.Activation, ET.DVE, ET.SP, ET.Pool}
    ctr = [0]
    for bass_bb in nc.bb_map.values():
        bb = bass_bb.bb
        il = bb.instructions
        out = []
        changed = False
        for inst in il:
            si = inst.sync_info
            waits = list(si.on_wait) if (si and si.on_wait) else []
            if len(waits) > _ENGINE_WAIT_LIMIT and inst.engine in _SPLIT_ENGINES:
                head, keep = waits[:-_ENGINE_WAIT_LIMIT], waits[-_ENGINE_WAIT_LIMIT:]
                for i in range(0, len(head), _ENGINE_WAIT_LIMIT):
                    nop = mybir.InstNoOp(name=f"waitsplit-{ctr[0]}", ins=[], outs=[])
                    ctr[0] += 1
                    nop.engine = inst.engine
                    nop.sync_info = mybir.SyncInfo(
                        on_wait=head[i : i + _ENGINE_WAIT_LIMIT], on_update=[]
                    )
                    nc.register_instruction(nop, overwrite=True)
                    out.append(nop)
                inst.sync_info = mybir.SyncInfo(
                    on_wait=keep, on_update=list(si.on_update or [])
                )
                changed = True
            out.append(inst)
        if changed:
            bb.instructions = out


def _split_drain_and_barrier(self, tick_clock, wait_clock):
    nc = self.nc
    _split_excess_waits(nc)
    drain_inst = nc.sync.drain()
    wait_clock.add_sem_waits(
        drain_inst.ins, ScopedClock({None: tick_clock.global_clock})
    )
    mi = drain_inst.ins
    waits = list(mi.sync_info.on_wait) if (mi.sync_info and mi.sync_info.on_wait) else []
    if len(waits) > _MAX_WAITS:
        upd = list(mi.sync_info.on_update) if mi.sync_info.on_update else []
        mi.sync_info = mybir.SyncInfo(on_wait=waits[:_MAX_WAITS], on_update=upd)
        for i in range(_MAX_WAITS, len(waits), _MAX_WAITS):
            nop = nc.sync.nop(nofuse=True)
            nop.ins.sync_info = mybir.SyncInfo(
                on_wait=waits[i : i + _MAX_WAITS], on_update=[]
            )
    nc.all_engine_barrier()
    assert self.sems is not None
    popped = nc._tile_sem_poison_stack.pop()
    assert popped is self._sem_poison
    nc.clear_and_free_semaphores(list(self.sems.allocated().values()))
    nc.all_engine_barrier()


tile_mod.TileContext._drain_and_barrier = _split_drain_and_barrier


# ---------------------------------------------------------------------------
# Host-side graph partitioning / layout


def _pack_core(node_ids, degs, nt, cap_e):
    """LPT: place nodes (descending degree) onto the least-edge-loaded tile
    that still has node capacity. Returns per-tile node-id arrays, or None
    if some tile exceeds cap_e edges."""
    order = np.argsort(-degs, kind="stable")
    tiles_n = [[] for _ in range(nt)]
    tile_ncnt = np.zeros(nt, np.int64)
    tile_ecnt = np.zeros(nt, np.int64)
    for j in order:
        cand = np.where(tile_ncnt < 128)[0]
        if len(cand) == 0:
            return None
        t = cand[np.argmin(tile_ecnt[cand])]
        tiles_n[t].append(node_ids[j])
        tile_ncnt[t] += 1
        tile_ecnt[t] += degs[j]
    if (tile_ecnt > cap_e).any():
        return None
    return [np.array(t, dtype=np.int64) for t in tiles_n]


def _prepare(inputs):
    nf = np.asarray(inputs["node_feats"], np.float32)
    ef = np.asarray(inputs["edge_feats"], np.float32)
    glob = np.asarray(inputs["globals_"], np.float32)
    recv = np.asarray(inputs["receivers"]).astype(np.int64)
    ngraph = np.asarray(inputs["node_graph"]).astype(np.int64)

    cnt = np.bincount(recv, minlength=N_NODES).astype(np.int64)
    egraph = ngraph[recv]
    ncnt_g = np.bincount(ngraph, minlength=N_GRAPHS)
    ecnt_g = np.bincount(egraph, minlength=N_GRAPHS)

    # pair graphs big+small by edge count to balance cores
    order = np.argsort(-ecnt_g, kind="stable")
    pairs = [(int(order[i]), int(order[N_GRAPHS - 1 - i])) for i in range(N_CORES)]

    core_nodes = [
        np.where((ngraph == pa) | (ngraph == pb))[0] for pa, pb in pairs
    ]
    NT = int(max((len(cn) + 127) // 128 for cn in core_nodes))

    core_of_graph = np.zeros(N_GRAPHS, np.int64)
    for c, (pa, pb) in enumerate(pairs):
        core_of_graph[pa] = c
        core_of_graph[pb] = c
    edge_core = core_of_graph[egraph]
    ecnt_core = np.bincount(edge_core, minlength=N_CORES)

    packs = None
    K0 = max(1, int(max(ecnt_core) + NT * 128 - 1) // (NT * 128))
    K0 = ((K0 + 3) // 4) * 4  # quad-chunk device loops need K0 % 4 == 0
    for k0 in range(K0, K0 + 12, 4):
        trial = []
        ok = True
        for c in range(N_CORES):
            p = _pack_core(core_nodes[c], cnt[core_nodes[c]], NT, k0 * 128)
            if p is None:
                ok = False
                break
            trial.append(p)
        if ok:
            packs, K0 = trial, k0
            break
    assert packs is not None, "bin packing failed"

    NPAD = NT * 128
    EPAD = NT * K0 * 128

    # replicated weights
    w_np = {}

    def bf(x):
        return np.ascontiguousarray(x).astype(npbf16)

    We1T = np.zeros((128, 256), np.float32)
    We1T[:32] = np.asarray(inputs["We1"], np.float32).T
    We1T[32] = np.asarray(inputs["be1"], np.float32)
    w_np["We1T"] = bf(We1T)

    We2T = np.asarray(inputs["We2"], np.float32).T  # [256, 128]
    w_np["We2T"] = bf(np.concatenate([We2T[:128], We2T[128:]], axis=1))  # [128,256]

    Wn1T = np.zeros((128, 256), np.float32)
    Wn1T[:64] = np.asarray(inputs["Wn1"], np.float32).T
    Wn1T[64] = np.asarray(inputs["bn1"], np.float32)
    w_np["Wn1T"] = bf(Wn1T)

    Win1T = np.asarray(inputs["Win1"], np.float32).T  # [256 fi, 256 fo]
    w_np["Win1T"] = bf(
        np.concatenate(
            [Win1T[:128, :128], Win1T[:128, 128:], Win1T[128:, :128], Win1T[128:, 128:]],
            axis=1,
        )
    )  # [128, 512]

    Wn2T = np.asarray(inputs["Wn2"], np.float32).T  # [256, 128]
    w_np["Wn2T"] = bf(np.concatenate([Wn2T[:128], Wn2T[128:]], axis=1))  # [128,256]
    w_np["Win2T"] = bf(np.asarray(inputs["Win2"], np.float32).T)  # [128, 128]

    WggT_pad = np.zeros((128, 128), np.float32)
    WggT_pad[:16] = np.asarray(inputs["Wgg"], np.float32).T
    w_np["WgnT"] = np.asarray(inputs["Wgn"], np.float32).T.copy()
    w_np["WgeT"] = np.asarray(inputs["Wge"], np.float32).T.copy()
    w_np["WggT"] = WggT_pad
    bgr_pad = np.zeros((128, 128), np.float32)
    bgr_pad[0] = np.asarray(inputs["bg"], np.float32)
    w_np["bgr"] = bgr_pad
    onesc = np.zeros((128, 2), np.float32)
    onesc[0] = 1.0
    w_np["onesc"] = onesc
    w_np["identb"] = np.eye(128, dtype=npbf16)
    w_np["ident4"] = np.eye(4, dtype=np.float32)

    Wg2 = np.asarray(inputs["Wg2"], np.float32)  # [128, 16]
    Wng2 = np.asarray(inputs["Wng2"], np.float32)
    be2 = np.asarray(inputs["be2"], np.float32)
    bn2 = np.asarray(inputs["bn2"], np.float32)

    # per-core inputs
    in_maps = []
    slot_of_node = np.full(N_NODES, -1, np.int64)
    tile_of_node = np.full(N_NODES, -1, np.int64)
    for c in range(N_CORES):
        pa, pb = pairs[c]
        for t in range(NT):
            ids = packs[c][t]
            slot_of_node[ids] = t * 128 + np.arange(len(ids))
            tile_of_node[ids] = t

        # ---- edges
        eidx = np.where(edge_core == c)[0]
        et = tile_of_node[recv[eidx]]
        order_e = np.argsort(et, kind="stable")
        eidx = eidx[order_e]
        et = et[order_e]
        counts = np.bincount(et, minlength=NT)
        starts = np.concatenate([[0], np.cumsum(counts)[:-1]])
        off_in = np.arange(len(eidx)) - np.repeat(starts, counts)
        dst = et * (K0 * 128) + off_in
        assert (counts <= K0 * 128).all()

        eftT = np.zeros((128, EPAD), np.float32)
        eftT[:32, dst] = ef[eidx].T
        eftT[32, dst] = 1.0
        eg = egraph[eidx]
        eftT[33, dst] = (eg == pa)
        eftT[34, dst] = (eg == pb)

        # one-hot: per 128-edge chunk a [lane, slot] block, K(partitions)=lanes
        onehot = np.zeros((128, EPAD), np.float32)
        onehot[dst % 128, (dst // 128) * 128 + slot_of_node[recv[eidx]] % 128] = 1.0

        # ---- nodes
        slot_node = np.full(NPAD, -1, np.int64)
        for t in range(NT):
            ids = packs[c][t]
            slot_node[t * 128 : t * 128 + len(ids)] = ids
        valid = slot_node >= 0
        sn = np.where(valid, slot_node, 0)

        nftT = np.zeros((128, NPAD), np.float32)
        nftT[:64, valid] = nf[sn[valid]].T
        nftT[64, valid] = 1.0
        ng = ngraph[sn]
        nftT[65] = valid * (ng == pa)
        nftT[66] = valid * (ng == pb)

        invc = np.ones((NPAD, 1), np.float32)
        invc[valid, 0] = 1.0 / np.maximum(cnt[sn[valid]], 1)

        poolw = np.zeros((NPAD, 128), np.float32)
        for g, gid in enumerate((pa, pb)):
            m = valid & (ng == gid)
            poolw[m, g] = 1.0 / max(ncnt_g[gid], 1)
            poolw[m, 2 + g] = cnt[sn[m]] / max(ecnt_g[gid], 1)

        g2aug = np.zeros((128, 128), np.float32)
        g2aug[32] = be2
        g2aug[33] = Wg2 @ glob[pa]
        g2aug[34] = Wg2 @ glob[pb]

        gnaug = np.zeros((128, 128), np.float32)
        gnaug[64] = bn2
        gnaug[65] = Wng2 @ glob[pa]
        gnaug[66] = Wng2 @ glob[pb]

        globT = np.zeros((128, 2), np.float32)
        globT[:16, 0] = glob[pa]
        globT[:16, 1] = glob[pb]

        m = {
            "eft": bf(eftT),
            "onehot": np.ascontiguousarray(onehot).astype(npfp8),
            "nft": bf(nftT),
            "invc": invc,
            "poolw": bf(poolw),
            "g2aug": bf(g2aug),
            "gnaug": bf(gnaug),
            "globT": globT,
        }
        m.update(w_np)
        in_maps.append(m)

    return in_maps, NT, K0, pairs


# ---------------------------------------------------------------------------
# Device program (identical on all cores)


def _build(NT, K0):
    nc = bass.Bass()
    NPAD = NT * 128
    EPAD = NT * K0 * 128
    CW = K0 * 128  # edge columns per node-tile

    d_eft = nc.dram_tensor("eft", [128, EPAD], BF16, kind="ExternalInput")
    d_onehot = nc.dram_tensor("onehot", [128, EPAD], FP8, kind="ExternalInput")
    d_nft = nc.dram_tensor("nft", [128, NPAD], BF16, kind="ExternalInput")
    d_invc = nc.dram_tensor("invc", [NPAD, 1], F32, kind="ExternalInput")
    d_poolw = nc.dram_tensor("poolw", [NPAD, 128], BF16, kind="ExternalInput")
    d_g2aug = nc.dram_tensor("g2aug", [128, 128], BF16, kind="ExternalInput")
    d_gnaug = nc.dram_tensor("gnaug", [128, 128], BF16, kind="ExternalInput")
    d_globT = nc.dram_tensor("globT", [128, 2], F32, kind="ExternalInput")

    d_We1T = nc.dram_tensor("We1T", [128, 256], BF16, kind="ExternalInput")
    d_We2T = nc.dram_tensor("We2T", [128, 256], BF16, kind="ExternalInput")
    d_Wn1T = nc.dram_tensor("Wn1T", [128, 256], BF16, kind="ExternalInput")
    d_Win1T = nc.dram_tensor("Win1T", [128, 512], BF16, kind="ExternalInput")
    d_Wn2T = nc.dram_tensor("Wn2T", [128, 256], BF16, kind="ExternalInput")
    d_Win2T = nc.dram_tensor("Win2T", [128, 128], BF16, kind="ExternalInput")
    d_WgnT = nc.dram_tensor("WgnT", [128, 128], F32, kind="ExternalInput")
    d_WgeT = nc.dram_tensor("WgeT", [128, 128], F32, kind="ExternalInput")
    d_WggT = nc.dram_tensor("WggT", [128, 128], F32, kind="ExternalInput")
    d_bgr = nc.dram_tensor("bgr", [128, 128], F32, kind="ExternalInput")
    d_onesc = nc.dram_tensor("onesc", [128, 2], F32, kind="ExternalInput")
    d_identb = nc.dram_tensor("identb", [128, 128], BF16, kind="ExternalInput")
    d_ident4 = nc.dram_tensor("ident4", [4, 4], F32, kind="ExternalInput")

    d_out = nc.dram_tensor("out", [128, 2], F32, kind="ExternalOutput")

    Relu = mybir.ActivationFunctionType.Relu
    Copy = mybir.ActivationFunctionType.Copy
    NQ = K0 // 2   # e1 chunk-pairs per tile
    NR = K0 // 4   # e2 quads per tile
    NS = K0 // 2   # e1T 256-edge slabs per tile

    with tile.TileContext(nc) as tc:
        with tc.tile_pool(name="wp", bufs=1) as wp:
            def wtile(dram, shape, dt):
                t = wp.tile(shape, dt, tag=dram.name)
                nc.gpsimd.dma_start(t[:], dram[:])
                return t

            We1T = wtile(d_We1T, [128, 256], BF16)
            We2T = wtile(d_We2T, [128, 256], BF16)
            Wn1T = wtile(d_Wn1T, [128, 256], BF16)
            Win1T = wtile(d_Win1T, [128, 512], BF16)
            Wn2T = wtile(d_Wn2T, [128, 256], BF16)
            Win2T = wtile(d_Win2T, [128, 128], BF16)
            g2aug = wtile(d_g2aug, [128, 128], BF16)
            gnaug = wtile(d_gnaug, [128, 128], BF16)
            WgnT = wtile(d_WgnT, [128, 128], F32)
            WgeT = wtile(d_WgeT, [128, 128], F32)
            WggT = wtile(d_WggT, [128, 128], F32)
            bgr = wtile(d_bgr, [128, 128], F32)
            onesc = wtile(d_onesc, [128, 2], F32)
            identb = wtile(d_identb, [128, 128], BF16)
            ident4 = wtile(d_ident4, [4, 4], F32)
            globT = wtile(d_globT, [128, 2], F32)

            aggall = wp.tile([128, 384 * NT], BF16, tag="aggall")

            # ----------------- edge phase -----------------
            with tc.tile_pool(name="ep", bufs=3) as ep, \
                 tc.tile_pool(name="e1p", bufs=2) as e1p, \
                 tc.tile_pool(name="efp", bufs=NQ + 2) as efp, \
                 tc.tile_pool(name="psA", bufs=2, space=bass.MemorySpace.PSUM) as psA, \
                 tc.tile_pool(name="psB", bufs=2, space=bass.MemorySpace.PSUM) as psB, \
                 tc.tile_pool(name="psC", bufs=2, space=bass.MemorySpace.PSUM) as psC, \
                 tc.tile_pool(name="psAgg", bufs=2, space=bass.MemorySpace.PSUM) as psAgg:
                for t in range(NT):
                    eftt = ep.tile([128, CW], BF16, tag="eftt", bufs=2)
                    nc.sync.dma_start(eftt[:], d_eft[:, t * CW : (t + 1) * CW])
                    oht = ep.tile([128, CW], FP8, tag="oht", bufs=2)
                    nc.sync.dma_start(oht[:], d_onehot[:, t * CW : (t + 1) * CW])
                    invc_t = ep.tile([128, 1], F32, tag="invc")
                    nc.sync.dma_start(invc_t[:], d_invc[t * 128 : (t + 1) * 128, :])

                    # e1T: feat-major halves, 256-edge slabs
                    e1h0 = e1p.tile([128, CW], BF16, tag="e1h0")
                    e1h1 = e1p.tile([128, CW], BF16, tag="e1h1")
                    for s in range(NS):
                        sl = slice(s * 256, (s + 1) * 256)
                        pT = psA.tile([128, 512], F32, tag="pT")
                        nc.tensor.matmul(pT[:, 0:256], We1T[:, 0:128],
                                         eftt[:, sl], start=True, stop=True)
                        nc.tensor.matmul(pT[:, 256:512], We1T[:, 128:256],
                                         eftt[:, sl], start=True, stop=True)
                        nc.scalar.activation(e1h0[:, sl], pT[:, 0:256], Relu)
                        nc.vector.tensor_scalar_max(e1h1[:, sl], pT[:, 256:512], 0.0)

                    # e1 edge-major, 2-chunk pairs -> ef cols {0:256, 384:640}
                    efs = []
                    for q in range(NQ):
                        pE = psB.tile([128, 512], F32, tag="pE")
                        for j in (0, 1):
                            c = 2 * q + j
                            nc.tensor.matmul(pE[:, j * 256 : (j + 1) * 256],
                                             eftt[:, c * 128 : (c + 1) * 128],
                                             We1T[:], start=True, stop=True)
                        ef = efp.tile([128, 768], FP8, tag="ef")
                        dst = ef[:].rearrange("p (j x) -> p j x", j=2)[:, :, 0:256]
                        src = pE[:].rearrange("p (j x) -> p j x", j=2)
                        if q % 2 == 0:
                            nc.scalar.activation(dst, src, Relu)
                        else:
                            nc.vector.tensor_scalar_max(dst, src, 0.0)
                        efs.append(ef)

                    # e2 edge-major, 4-chunk quads -> ef cols {256:384, 640:768}
                    for r in range(NR):
                        pC = psC.tile([128, 512], F32, tag="pC")
                        for j in range(4):
                            c = 4 * r + j
                            csl = slice(c * 128, (c + 1) * 128)
                            osl = slice(j * 128, (j + 1) * 128)
                            nc.tensor.matmul(pC[:, osl], e1h0[:, csl],
                                             We2T[:, 0:128], start=True, stop=False)
                            nc.tensor.matmul(pC[:, osl], e1h1[:, csl],
                                             We2T[:, 128:256], start=False, stop=False)
                            nc.tensor.matmul(pC[:, osl], eftt[:, csl],
                                             g2aug[:], start=False, stop=True)
                        for h in (0, 1):
                            ef = efs[2 * r + h]
                            dst = ef[:].rearrange("p (j x) -> p j x", j=2)[:, :, 256:384]
                            src = pC[:, h * 256 : (h + 1) * 256].rearrange(
                                "p (j x) -> p j x", j=2)
                            if h == 0:
                                nc.scalar.activation(dst, src, Relu)
                            else:
                                nc.vector.tensor_scalar_max(dst, src, 0.0)

                    # aggregation: fp8 DoubleRow, 256 edges (2 chunks) per matmul
                    pagg = psAgg.tile([128, 384], F32, tag="pagg")
                    for q in range(NQ):
                        lhs3 = oht[:, q * 256 : (q + 1) * 256].rearrange(
                            "k (p m) -> k p m", p=2)
                        rhs3 = efs[q][:].rearrange("k (p n) -> k p n", p=2)
                        nc.tensor.matmul(pagg[:], lhs3, rhs3,
                                         start=(q == 0), stop=(q == NQ - 1),
                                         perf_mode=mybir.MatmulPerfMode.DoubleRow)

                    nc.scalar.activation(
                        aggall[:, t * 384 : (t + 1) * 384], pagg[:], Copy,
                        scale=invc_t[:],
                    )

            # ----------------- node phase -----------------
            with tc.tile_pool(name="np_", bufs=NT) as np_, \
                 tc.tile_pool(name="agp", bufs=NT) as agp, \
                 tc.tile_pool(name="nsb", bufs=3) as nsb, \
                 tc.tile_pool(name="npsT", bufs=2, space=bass.MemorySpace.PSUM) as npsT, \
                 tc.tile_pool(name="npsB", bufs=2, space=bass.MemorySpace.PSUM) as npsB, \
                 tc.tile_pool(name="npsC", bufs=1, space=bass.MemorySpace.PSUM) as npsC, \
                 tc.tile_pool(name="npsP", bufs=1, space=bass.MemorySpace.PSUM) as npsP:
                ppN = npsP.tile([128, 128], F32, tag="ppN")
                ppE = npsP.tile([128, 128], F32, tag="ppE")

                nftts, pws = [], []
                for t in range(NT):
                    nftt = np_.tile([128, 128], BF16, tag="nftt")
                    nc.gpsimd.dma_start(nftt[:], d_nft[:, t * 128 : (t + 1) * 128])
                    pw = np_.tile([128, 128], BF16, tag="pw")
                    nc.gpsimd.dma_start(pw[:], d_poolw[t * 128 : (t + 1) * 128, :])
                    nftts.append(nftt)
                    pws.append(pw)

                # pass 1: transpose agg tiles to feat-major
                aggTs = []
                for t in range(NT):
                    aggsl = aggall[:, t * 384 : (t + 1) * 384]
                    pT = npsT.tile([128, 384], BF16, tag="pT")
                    nc.tensor.transpose(pT[:, 0:128], aggsl[:, 0:128], identb[:])
                    nc.tensor.transpose(pT[:, 128:256], aggsl[:, 128:256], identb[:])
                    nc.tensor.transpose(pT[:, 256:384], aggsl[:, 256:384], identb[:])
                    aggT = agp.tile([128, 384], BF16, tag="aggT")
                    nc.vector.tensor_copy(aggT[:], pT[:])
                    aggTs.append(aggT)

                # pass 2: node MLPs + pooling
                for t in range(NT):
                    aggsl = aggall[:, t * 384 : (t + 1) * 384]
                    aggT = aggTs[t]
                    nftt = nftts[t]
                    pw = pws[t]

                    pn1 = npsB.tile([128, 256], F32, tag="pn1")
                    for s in (0, 1):
                        ssl = slice(s * 128, (s + 1) * 128)
                        nc.tensor.matmul(pn1[:, ssl], Wn1T[:, ssl], nftt[:],
                                         start=True, stop=False)
                        nc.tensor.matmul(pn1[:, ssl], Win1T[:, s * 128 : s * 128 + 128],
                                         aggT[:, 0:128], start=False, stop=False)
                        nc.tensor.matmul(pn1[:, ssl], Win1T[:, 256 + s * 128 : 256 + s * 128 + 128],
                                         aggT[:, 128:256], start=False, stop=True)
                    n1T = nsb.tile([128, 256], BF16, tag="n1T")
                    nc.scalar.activation(n1T[:], pn1[:], Relu)

                    pn2 = npsC.tile([128, 128], F32, tag="pn2")
                    nc.tensor.matmul(pn2[:], n1T[:, 0:128], Wn2T[:, 0:128], start=True, stop=False)
                    nc.tensor.matmul(pn2[:], n1T[:, 128:256], Wn2T[:, 128:256], start=False, stop=False)
                    nc.tensor.matmul(pn2[:], aggT[:, 256:384], Win2T[:], start=False, stop=False)
                    nc.tensor.matmul(pn2[:], nftt[:], gnaug[:], start=False, stop=True)
                    n2 = nsb.tile([128, 128], BF16, tag="n2")
                    nc.vector.tensor_scalar_max(n2[:], pn2[:], 0.0)

                    nc.tensor.matmul(ppN[:], pw[:], n2[:],
                                     start=(t == 0), stop=(t == NT - 1))
                    nc.tensor.matmul(ppE[:], pw[:], aggsl[:, 256:384],
                                     start=(t == 0), stop=(t == NT - 1))

                # ----------------- final projection -----------------
                pp4 = nsb.tile([4, 256], F32, tag="pp4")
                nc.scalar.activation(pp4[:, 0:128], ppN[0:4, :], Copy)
                nc.scalar.activation(pp4[:, 128:256], ppE[0:4, :], Copy)

                ptail = npsP.tile([128, 16], F32, tag="ptail")
                nc.tensor.transpose(ptail[:, 0:4], pp4[:, 0:128], ident4[:])
                nc.tensor.transpose(ptail[:, 4:8], pp4[:, 128:256], ident4[:])
                nt8 = nsb.tile([128, 8], F32, tag="nt8")
                nc.scalar.activation(nt8[:], ptail[:, 0:8], Copy)

                nc.tensor.matmul(ptail[:, 8:10], WgnT[:], nt8[:, 0:2], start=True, stop=False)
                nc.tensor.matmul(ptail[:, 8:10], WgeT[:], nt8[:, 6:8], start=False, stop=False)
                nc.tensor.matmul(ptail[:, 8:10], WggT[:], globT[:], start=False, stop=False)
                nc.tensor.matmul(ptail[:, 8:10], bgr[:], onesc[:], start=False, stop=True)
                outsb = nsb.tile([128, 2], F32, tag="outsb")
                nc.scalar.activation(outsb[:], ptail[:, 8:10], Copy)
                nc.sync.dma_start(d_out[:], outsb[:])

    return nc


_CACHE = {}


def _get_nc(NT, K0):
    key = (NT, K0)
    if key not in _CACHE:
        _CACHE[key] = _build(NT, K0)
    return _CACHE[key]


def _run(inputs, trace=False):
    in_maps, NT, K0, pairs = _prepare(inputs)
    nc = _get_nc(NT, K0)
    res = run_bass_kernel_spmd(nc, in_maps, list(range(N_CORES)), trace=trace)
    out = np.zeros((N_GRAPHS, 128), np.float32)
    for c in range(N_CORES):
        r = np.asarray(res.results[c]["out"], np.float32)
        pa, pb = pairs[c]
        out[pa] = r[:, 0]
        out[pb] = r[:, 1]
    return out, res


def kernel(**inputs):
    out, _ = _run(inputs, trace=False)
    return out


def kernel_traced(**inputs):
    return _run(inputs, trace=True)
